# revision 26
# baseline (speedup 1.0000x reference)
"""Trainium2 Bass kernel for nn_AutopoieticEngine (scatter_memory).

Self-contained: takes FULL inputs (as produced by the problem's
setup_inputs), shards the cell dimension across 8 NeuronCores, runs a
Bass/Tile kernel per core (gather -> MLP+GRU -> faction sync/debate ->
scatter), all-reduces the tiny faction/softmax statistics on-device, and
reassembles the full outputs on the host.

Sharding: alive positions split into 8 chunks of 16384 = exactly one
faction per core (Na=131072, n_f=8, fs=16384).  Core k owns hiddens rows
[c_k, c_{k+1}) where c_k = alive_idx[16384*k], so its scatter targets are
entirely inside its own (padded) row slice.
"""

import numpy as np

try:
    import ml_dtypes

    BF16 = ml_dtypes.bfloat16
except Exception:  # pragma: no cover
    BF16 = None

CORES = 8
H = 128          # hidden dim
IND = 64         # input dim
D = 64           # out dim
N_CELLS = 262144
NA = 131072
PC = NA // CORES      # alive positions per core == faction size
S = 33792             # padded hiddens-slice rows per core (264*128)
HALF = 8192           # int16-index base split point (positions per core)
NBLK = 8              # cell blocks per core
BC = PC // NBLK       # 2048 cells per block
NCH = BC // 512       # 512-wide matmul chunks per block
CH = 512
SYNC = 0.15
DEBATE = 0.15
CPR = 4224            # copy-pass rows per tile (S/8, multiple of 128)

_CACHE = {}


def _variant():
    import os
    return frozenset(
        v for v in os.environ.get("KVAR", "").split(",") if v)


# --------------------------------------------------------------------------
# numpy fallback (exact reference semantics) for inputs that violate the
# layout assumptions this kernel hardcodes.
# --------------------------------------------------------------------------
def _np_reference(x, hiddens, Wa1, ba1, Wa2, ba2, Wg1, bg1, Wg2, bg2,
                  W_ih, W_hh, b_ih, b_hh, alive_idx, step):
    idx = np.asarray(alive_idx).astype(np.int64)
    h = hiddens[idx]
    xb = np.broadcast_to(x, (h.shape[0], x.shape[-1]))
    c = np.concatenate([xb, h], axis=-1)
    a = np.maximum(c @ Wa1.T + ba1, 0) @ Wa2.T + ba2
    g = np.maximum(c @ Wg1.T + bg1, 0) @ Wg2.T + bg2
    out = a - g
    tension = np.mean(out * out, axis=-1, keepdims=True)
    mem_in = np.concatenate([out, tension], axis=-1)
    gi = mem_in @ W_ih.T + b_ih
    gh = h @ W_hh.T + b_hh
    Hd = h.shape[-1]
    ir, iz, inn = gi[:, :Hd], gi[:, Hd:2 * Hd], gi[:, 2 * Hd:]
    hr, hz, hn = gh[:, :Hd], gh[:, Hd:2 * Hd], gh[:, 2 * Hd:]
    r = 1.0 / (1.0 + np.exp(-(ir + hr)))
    z = 1.0 / (1.0 + np.exp(-(iz + hz)))
    nn_ = np.tanh(inn + r * hn)
    new_h = (1.0 - z) * nn_ + z * h
    n, Hh = new_h.shape
    n_f = min(8, n // 2)
    if n_f >= 2:
        fs = n // n_f
        hb = new_h[: n_f * fs].reshape(n_f, fs, Hh)
        fm = hb.mean(axis=1, keepdims=True)
        hb = (1.0 - SYNC) * hb + SYNC * fm
        if step > 5:
            go = hb.mean(axis=1).mean(axis=0)
            dc = max(1, fs // 4)
            hb[:, :dc] = (1.0 - DEBATE) * hb[:, :dc] + DEBATE * go
        new_h = np.concatenate([hb.reshape(n_f * fs, Hh), new_h[n_f * fs:]], 0)
    new_hiddens = np.asarray(hiddens).copy()
    new_hiddens[idx] = new_h
    t = tension[:, 0]
    tm = t.max()
    w = np.exp(t - tm)
    w = w / w.sum()
    combined = (w[:, None] * out).sum(axis=0, keepdims=True)
    mean_tension = np.float32(t.mean())
    return (combined.astype(np.float32), mean_tension,
            new_hiddens.astype(np.float32))


# --------------------------------------------------------------------------
# graph builder
# --------------------------------------------------------------------------
def _build_graph(step_gt5: bool):
    import concourse.bass as bass
    import concourse.mybir as mybir
    import concourse.tile as tile
    from concourse import bacc
    from concourse.masks import make_identity

    var = _variant()

    f32 = mybir.dt.float32
    bf16 = mybir.dt.bfloat16
    i16 = mybir.dt.int16
    Alu = mybir.AluOpType
    Act = mybir.ActivationFunctionType

    nc = bacc.Bacc("TRN2", target_bir_lowering=False, debug=False,
                   num_devices=CORES)

    hslf = nc.declare_dram_parameter("hslf", [S, H], f32, isOutput=False)
    hslb = nc.declare_dram_parameter("hslb", [S, H], bf16, isOutput=False)
    idxw = nc.declare_dram_parameter("idxw", [128, NBLK * (BC // 16)], i16,
                                     isOutput=False)
    wbl = nc.declare_dram_parameter("wbl", [128, 1280], bf16, isOutput=False)
    bbl = nc.declare_dram_parameter("bbl", [128, 8], f32, isOutput=False)
    outsl = nc.declare_dram_parameter("outsl", [S, H], f32, isOutput=True)
    osmall = nc.declare_dram_parameter("osmall", [128, 4], f32, isOutput=True)

    arin = nc.dram_tensor("arin", [128, 4], f32)
    arout = nc.dram_tensor("arout", [128, 4], f32, addr_space="Shared")

    # constants for the fused sync/debate delta:
    #   delta = alpha*nn + beta*zd - d + v
    # non-debate: alpha=-SYNC, beta=1-SYNC, v=SYNC*fm
    # debate:     a2=(1-SYNC)(1-DEBATE); alpha=a2-1, beta=a2,
    #             v=SYNC*(1-DEBATE)*fm + DEBATE*go
    a2 = (1.0 - SYNC) * (1.0 - DEBATE)

    with tile.TileContext(nc) as tc:
        with (
            tc.tile_pool(name="const", bufs=1) as cpool,
            tc.tile_pool(name="per", bufs=1) as per,
            tc.tile_pool(name="ablk", bufs=1) as ablk,
            tc.tile_pool(name="h2", bufs=2) as h2,
            tc.tile_pool(name="bblk", bufs=2) as bblk,
            tc.tile_pool(name="cp", bufs=2) as cp,
            tc.tile_pool(name="ps", bufs=1, space="PSUM") as _ps0,
            tc.tile_pool(name="ps2", bufs=2, space="PSUM") as _ps2,
            tc.tile_pool(name="ps3", bufs=2, space="PSUM") as _ps3,
        ):
            HB = BC // 2  # 1024: half-block, one wide-psum tile
            # ---------------- constant / persistent tiles ----------------
            wsb = cpool.tile([128, 1280], bf16)
            bsb = cpool.tile([128, 8], f32)
            idxsb = cpool.tile([128, NBLK * (BC // 16)], i16)
            ident = cpool.tile([128, 128], bf16)
            ones_row = cpool.tile([1, CH], bf16)      # rhs for b_hn outer
            ones64 = cpool.tile([64, 65], bf16)       # tension lhsT (1/64)
            ones_t = cpool.tile([65, 64], bf16)       # e-bcast lhsT @ base 64

            nc.sync.dma_start(out=wsb[:], in_=wbl[:, :])
            nc.sync.dma_start(out=bsb[:], in_=bbl[:, :])
            nc.sync.dma_start(out=idxsb[:], in_=idxw[:, :])
            make_identity(nc, ident[:])
            nc.gpsimd.memset(ones_row[:], 1.0)
            nc.gpsimd.memset(ones64[:], 1.0 / 64.0)
            nc.gpsimd.memset(ones_t[:], 1.0)

            pa_cm = per.tile([128, PC], bf16)  # cell-major pa (transposed)
            mem = per.tile([65, PC], bf16)     # rows 0:64 out, row 64 t
            accn = per.tile([128, NBLK], f32)  # sum(nn) per block
            accz = per.tile([128, NBLK], f32)  # sum(z*d) per block
            acct = per.tile([65, NCH * NBLK], f32)  # row 64: sum(t) slots
            acce = per.tile([65, 1], f32)          # row 64: sum(e)
            wacc = per.tile([64, NCH * NBLK], f32)  # wout partials
            sums = per.tile([128, 4], f32)     # AllReduce payload
            arsb = per.tile([128, 4], f32)     # AllReduce result
            vvec = per.tile([128, 2], f32)     # v (plain, debate)
            tmpv = per.tile([128, 1], f32)
            tmpn = per.tile([128, 1], f32)
            v_cm_p = per.tile([128, 128], bf16)   # v broadcast, plain
            v_cm_d = per.tile([128, 128], bf16)   # v broadcast, debate
            vcol_bf = per.tile([128, 1], bf16)
            vrow_bf = per.tile([1, 128], bf16)

            nc.gpsimd.memset(sums[:], 0.0)

            # ---------------- pass-through copy of the slice --------------
            for i in range(S // CPR):
                ct = cp.tile([128, CPR], f32, tag="cp")
                src = hslf[i * CPR:(i + 1) * CPR, :].rearrange(
                    "(p a) d -> p (a d)", p=128)
                dst = outsl[i * CPR:(i + 1) * CPR, :].rearrange(
                    "(p a) d -> p (a d)", p=128)
                nc.sync.dma_start(out=ct[:], in_=src)
                nc.sync.dma_start(out=dst, in_=ct[:])

            # ---------------- phase A ------------------------------------
            def emit_gather(b, h_tile):
                if "nogather" in var:
                    nc.gpsimd.memset(h_tile[:], 0.02)
                    return
                in_ap = hslb[:, :] if b < NBLK // 2 else hslb[HALF:, :]
                # transpose-mode dma_gather is limited to 512 idxs/call
                for c in range(NCH):
                    nc.gpsimd.dma_gather(
                        h_tile[:, c * CH:(c + 1) * CH].rearrange(
                            "p (a n) -> p a n", a=1),
                        in_ap,
                        idxsb[:, (b * NCH + c) * (CH // 16):
                              (b * NCH + c + 1) * (CH // 16)],
                        CH,
                        CH,
                        H,
                        transpose=True,
                    )

            h_tiles = {}
            h_tiles[0] = h2.tile([128, BC], bf16, tag="h", name="hbf0")
            emit_gather(0, h_tiles[0])

            for b in range(NBLK):
                blk = slice(b * BC, (b + 1) * BC)
                h_bf = h_tiles.pop(b)
                if b + 1 < NBLK:
                    h_tiles[b + 1] = h2.tile([128, BC], bf16, tag="h", name=f"hbf{b+1}")
                    emit_gather(b + 1, h_tiles[b + 1])

                a1 = ablk.tile([128, BC], bf16, tag="a1")
                g1 = ablk.tile([128, BC], bf16, tag="g1")
                sq = ablk.tile([64, BC], bf16, tag="sq")
                r_bf = ablk.tile([128, BC], bf16, tag="r")
                z_bf = ablk.tile([128, BC], bf16, tag="z")
                rhn = ablk.tile([128, BC], bf16, tag="rhn")
                s_bf = ablk.tile([128, BC], bf16, tag="s")
                nn_bf = ablk.tile([128, BC], bf16, tag="nn")
                d_bf = ablk.tile([128, BC], bf16, tag="d")
                zd_bf = ablk.tile([128, BC], bf16, tag="zd")
                q1 = ablk.tile([128, BC], bf16, tag="q1")

                # stage 1/2: first-layer MLPs (x-part folded into bias)
                for c in range(NCH):
                    cs = slice(c * CH, (c + 1) * CH)
                    p1 = _ps2.tile([128, CH], f32, tag="p1")
                    nc.tensor.matmul(out=p1[:], lhsT=wsb[:, 0:128],
                                     rhs=h_bf[:, cs], start=True, stop=True)
                    nc.scalar.activation(a1[:, cs], p1[:], Act.Relu,
                                         bias=bsb[:, 0:1])
                for c in range(NCH):
                    cs = slice(c * CH, (c + 1) * CH)
                    p1 = _ps2.tile([128, CH], f32, tag="p1")
                    nc.tensor.matmul(out=p1[:], lhsT=wsb[:, 128:256],
                                     rhs=h_bf[:, cs], start=True, stop=True)
                    nc.scalar.activation(g1[:, cs], p1[:], Act.Relu,
                                         bias=bsb[:, 1:2])

                # stage 3: out = Wa2@a1 - Wg2@g1 + b_out  -> mem rows 0:64
                for c in range(NCH):
                    cs = slice(c * CH, (c + 1) * CH)
                    mcs = slice(b * BC + c * CH, b * BC + (c + 1) * CH)
                    po = _ps2.tile([64, CH], f32, tag="p1")
                    nc.tensor.matmul(out=po[:], lhsT=wsb[0:128, 256:320],
                                     rhs=a1[:, cs], start=True, stop=False)
                    nc.tensor.matmul(out=po[:], lhsT=wsb[0:128, 320:384],
                                     rhs=g1[:, cs], start=False, stop=True)
                    nc.scalar.activation(mem[0:64, mcs], po[:], Act.Identity,
                                         bias=bsb[0:64, 2:3])

                # tension: t = mean(out^2) over features -> mem row 64
                nc.vector.tensor_tensor(out=sq[:], in0=mem[0:64, blk],
                                        in1=mem[0:64, blk], op=Alu.mult)
                for c in range(NCH):
                    cs = slice(c * CH, (c + 1) * CH)
                    mcs = slice(b * BC + c * CH, b * BC + (c + 1) * CH)
                    pt = _ps2.tile([65, CH], f32, tag="p1")
                    nc.tensor.matmul(out=pt[:, :], lhsT=ones64[:, :],
                                     rhs=sq[:, cs], start=True, stop=True)
                    nc.scalar.activation(
                        mem[64:65, mcs], pt[64:65, :], Act.Copy,
                        accum_out=acct[64:65, b * NCH + c:b * NCH + c + 1])

                # GRU gates
                for c in range(NCH):
                    cs = slice(c * CH, (c + 1) * CH)
                    mcs = slice(b * BC + c * CH, b * BC + (c + 1) * CH)
                    pr = _ps0.tile([128, CH], f32, tag="pr")
                    pz = _ps0.tile([128, CH], f32, tag="pz")
                    pi = _ps0.tile([128, CH], f32, tag="pi")
                    ph = _ps0.tile([128, CH], f32, tag="ph")
                    nc.tensor.matmul(out=pr[:], lhsT=wsb[0:65, 768:896],
                                     rhs=mem[0:65, mcs], start=True,
                                     stop=False)
                    nc.tensor.matmul(out=pr[:], lhsT=wsb[:, 384:512],
                                     rhs=h_bf[:, cs], start=False, stop=True)
                    nc.tensor.matmul(out=pz[:], lhsT=wsb[0:65, 896:1024],
                                     rhs=mem[0:65, mcs], start=True,
                                     stop=False)
                    nc.tensor.matmul(out=pz[:], lhsT=wsb[:, 512:640],
                                     rhs=h_bf[:, cs], start=False, stop=True)
                    nc.tensor.matmul(out=pi[:], lhsT=wsb[0:65, 1024:1152],
                                     rhs=mem[0:65, mcs], start=True,
                                     stop=True)
                    nc.tensor.matmul(out=ph[:], lhsT=wsb[0:1, 1152:1280],
                                     rhs=ones_row[:, :], start=True,
                                     stop=False)
                    nc.tensor.matmul(out=ph[:], lhsT=wsb[:, 640:768],
                                     rhs=h_bf[:, cs], start=False, stop=True)
                    nc.scalar.activation(r_bf[:, cs], pr[:], Act.Sigmoid,
                                         bias=bsb[:, 3:4])
                    nc.scalar.activation(z_bf[:, cs], pz[:], Act.Sigmoid,
                                         bias=bsb[:, 4:5])
                    nc.vector.tensor_tensor(out=rhn[:, cs], in0=r_bf[:, cs],
                                            in1=ph[:], op=Alu.mult)
                    nc.vector.tensor_tensor(out=s_bf[:, cs], in0=rhn[:, cs],
                                            in1=pi[:], op=Alu.add)

                nc.scalar.activation(nn_bf[:], s_bf[:], Act.Tanh,
                                     bias=bsb[:, 5:6],
                                     accum_out=accn[:, b:b + 1])
                nc.vector.tensor_tensor(out=d_bf[:], in0=h_bf[:],
                                        in1=nn_bf[:], op=Alu.subtract)
                nc.vector.scalar_tensor_tensor(
                    out=zd_bf[:], in0=z_bf[:], scalar=1.0, in1=d_bf[:],
                    op0=Alu.mult, op1=Alu.mult,
                    accum_out=accz[:, b:b + 1])
                deb = step_gt5 and (b * BC < PC // 4)
                alpha = (a2 - 1.0) if deb else -SYNC
                beta = a2 if deb else (1.0 - SYNC)
                nc.vector.scalar_tensor_tensor(
                    out=q1[:], in0=zd_bf[:], scalar=beta, in1=d_bf[:],
                    op0=Alu.mult, op1=Alu.subtract)
                pa_b = ablk.tile([128, BC], bf16, tag="pab")
                nc.vector.scalar_tensor_tensor(
                    out=pa_b[:], in0=nn_bf[:], scalar=alpha, in1=q1[:],
                    op0=Alu.mult, op1=Alu.add)
                # transpose to cell-major now; v is added later in phase B
                for hh in range(2):
                    ptr = _ps3.tile([128, HB], bf16, tag="ptr")
                    for j in range(HB // 128):
                        col = hh * HB + j * 128
                        nc.tensor.transpose(
                            out=ptr[:, j * 128:(j + 1) * 128],
                            in_=pa_b[:, col:col + 128],
                            identity=ident[:])
                    dsl = slice(b * BC + hh * HB, b * BC + (hh + 1) * HB)
                    if (b + hh) % 2 == 0:
                        nc.scalar.activation(pa_cm[:, dsl], ptr[:], Act.Copy)
                    else:
                        nc.vector.tensor_copy(pa_cm[:, dsl], ptr[:])

            # ---------------- local stats (no AllReduce needed) -----------
            nc.vector.tensor_reduce(out=tmpn[:], in_=accn[:, :],
                                    axis=mybir.AxisListType.X, op=Alu.add)
            nc.vector.tensor_reduce(out=tmpv[:], in_=accz[:, :],
                                    axis=mybir.AxisListType.X, op=Alu.add)
            nc.vector.tensor_tensor(out=sums[:, 0:1], in0=tmpn[:],
                                    in1=tmpv[:], op=Alu.add)
            # v_plain = SYNC/PC * local_sum
            nc.vector.tensor_scalar(out=vvec[:, 0:1], in0=sums[:, 0:1],
                                    scalar1=SYNC / PC, scalar2=None,
                                    op0=Alu.mult)

            def emit_vcm(col, dst):
                # dst[i, j] = vvec[j, col] for all i (broadcast matrix)
                nc.vector.tensor_copy(vcol_bf[:], vvec[:, col:col + 1])
                pv1 = _ps0.tile([1, 128], f32, tag="pz", name=f"pv1_{col}")
                nc.tensor.matmul(out=pv1[:], lhsT=vcol_bf[:],
                                 rhs=ident[:], start=True, stop=True)
                nc.scalar.activation(vrow_bf[:], pv1[:], Act.Copy)
                pv2 = _ps0.tile([128, 128], f32, tag="pr", name=f"pv2_{col}")
                nc.tensor.matmul(out=pv2[:], lhsT=ones_row[0:1, 0:128],
                                 rhs=vrow_bf[:], start=True, stop=True)
                nc.scalar.activation(dst[:], pv2[:], Act.Copy)

            emit_vcm(0, v_cm_p)

            # ---------------- phase B ------------------------------------
            def emit_phase_b(b):
                deb = step_gt5 and (b * BC < PC // 4)
                vt = v_cm_d if deb else v_cm_p
                dcm = bblk.tile([128, BC], f32, tag="dcm")
                nc.vector.tensor_tensor(
                    out=dcm[:].rearrange("p (a n) -> p a n", n=H),
                    in0=pa_cm[:, b * BC:(b + 1) * BC].rearrange(
                        "p (a n) -> p a n", n=H),
                    in1=vt[:, None, :].to_broadcast([128, BC // H, H]),
                    op=Alu.add)
                if "noscatter" not in var:
                    out_ap = outsl[:, :] if b < NBLK // 2 \
                        else outsl[HALF:, :]
                    idx_ap = idxsb[:, b * (BC // 16):(b + 1) * (BC // 16)]
                    nc.gpsimd.dma_scatter_add(
                        out_ap,
                        dcm[:].rearrange("p (a n) -> p a n", n=H),
                        idx_ap,
                        BC,
                        BC,
                        H,
                    )

            for b in range(2, NBLK):
                emit_phase_b(b)

            # ---------------- global stats + AllReduce --------------------
            nc.vector.tensor_reduce(out=sums[64:65, 3:4],
                                    in_=acct[64:65, :],
                                    axis=mybir.AxisListType.X, op=Alu.add)
            # e = exp(t) in place on mem row 64 (t no longer needed)
            nc.scalar.activation(mem[64:65, :], mem[64:65, :], Act.Exp,
                                 accum_out=acce[64:65, 0:1])
            nc.vector.tensor_scalar(out=sums[64:65, 2:3],
                                    in0=acce[64:65, 0:1], scalar1=1.0,
                                    scalar2=None, op0=Alu.mult)
            # wout partials: e broadcast via PE, multiply-reduce on DVE
            for c in range(NCH * NBLK):
                cs = slice(c * CH, (c + 1) * CH)
                pe = _ps0.tile([64, CH], f32, tag="pz")
                nc.tensor.matmul(out=pe[:], lhsT=ones_t[64:65, 0:64],
                                 rhs=mem[64:65, cs], start=True, stop=True)
                eo = bblk.tile([64, CH], bf16, tag="eo")
                nc.vector.scalar_tensor_tensor(
                    out=eo[:], in0=mem[0:64, cs], scalar=1.0, in1=pe[:],
                    op0=Alu.mult, op1=Alu.mult,
                    accum_out=wacc[:, c:c + 1])
            nc.vector.tensor_reduce(out=sums[0:64, 1:2], in_=wacc[:, :],
                                    axis=mybir.AxisListType.X, op=Alu.add)

            if "nocoll" in var:
                nc.vector.tensor_scalar(out=arsb[:], in0=sums[:],
                                        scalar1=float(CORES), scalar2=None,
                                        op0=Alu.mult)
            else:
                nc.gpsimd.dma_start(out=arin[:, :], in_=sums[:])
                nc.gpsimd.collective_compute(
                    "AllReduce", Alu.add,
                    replica_groups=[list(range(CORES))],
                    ins=[arin.ap().opt()],
                    outs=[arout.ap().opt()],
                )
                nc.gpsimd.dma_start(out=arsb[:], in_=arout[:, :])
            nc.sync.dma_start(out=osmall[:, :], in_=arsb[:])

            # v_debate = SYNC*(1-DEBATE)/PC * local + DEBATE/NA * total
            nc.vector.tensor_scalar(out=tmpv[:], in0=sums[:, 0:1],
                                    scalar1=SYNC * (1.0 - DEBATE) / PC,
                                    scalar2=None, op0=Alu.mult)
            nc.vector.scalar_tensor_tensor(
                out=vvec[:, 1:2], in0=arsb[:, 0:1], scalar=DEBATE / NA,
                in1=tmpv[:], op0=Alu.mult, op1=Alu.add)
            emit_vcm(1, v_cm_d)

            for b in (0, 1):
                emit_phase_b(b)

    nc.compile()
    return nc


def _get_graph(step_gt5: bool):
    key = (bool(step_gt5), _variant())
    if key not in _CACHE:
        _CACHE[key] = _build_graph(step_gt5)
    return _CACHE[key]


# --------------------------------------------------------------------------
# host-side sharding + launch
# --------------------------------------------------------------------------
def _wrap_idx(vals):
    """int16 index layout for dma_gather/dma_scatter_add: [128, n//16],
    idx q stored at [q % 16, q // 16], replicated to all 8 Q7 groups."""
    n = vals.shape[0]
    w = vals.reshape(n // 16, 16).T.astype(np.int16)    # [16, n//16]
    return np.tile(w, (8, 1))                            # [128, n//16]


def kernel(**inputs):
    x = np.asarray(inputs["x"], np.float32)
    hiddens = np.asarray(inputs["hiddens"], np.float32)
    Wa1 = np.asarray(inputs["Wa1"], np.float32)
    ba1 = np.asarray(inputs["ba1"], np.float32)
    Wa2 = np.asarray(inputs["Wa2"], np.float32)
    ba2 = np.asarray(inputs["ba2"], np.float32)
    Wg1 = np.asarray(inputs["Wg1"], np.float32)
    bg1 = np.asarray(inputs["bg1"], np.float32)
    Wg2 = np.asarray(inputs["Wg2"], np.float32)
    bg2 = np.asarray(inputs["bg2"], np.float32)
    W_ih = np.asarray(inputs["W_ih"], np.float32)
    W_hh = np.asarray(inputs["W_hh"], np.float32)
    b_ih = np.asarray(inputs["b_ih"], np.float32)
    b_hh = np.asarray(inputs["b_hh"], np.float32)
    ai = np.asarray(inputs["alive_idx"]).astype(np.int64)
    step = int(inputs["step"])

    ok = (
        BF16 is not None
        and hiddens.shape == (N_CELLS, H)
        and ai.shape == (NA,)
        and np.all(np.diff(ai) > 0)
        and step > 5
    )
    if ok:
        c = [0] + [int(ai[PC * k]) for k in range(1, CORES)] + [N_CELLS]
        lens = [c[k + 1] - c[k] for k in range(CORES)]
        for k in range(CORES):
            li = ai[PC * k:PC * (k + 1)] - c[k]
            if lens[k] > S or li[:HALF].max() > 32000 or \
               li[HALF:].min() < HALF or (li[HALF:] - HALF).max() > S - HALF - 1:
                ok = False
                break
    if not ok:
        return _np_reference(x, hiddens, Wa1, ba1, Wa2, ba2, Wg1, bg1, Wg2,
                             bg2, W_ih, W_hh, b_ih, b_hh, ai, step)

    # ---- replicated weight prep (x-part of layer-1 folded into biases) ----
    ba1e = ba1 + Wa1[:, :IND] @ x[0]
    bg1e = bg1 + Wg1[:, :IND] @ x[0]
    b_out = ba2 - bg2
    b_rz = (b_ih + b_hh)[:2 * H]
    b_in = b_ih[2 * H:]
    b_hn = b_hh[2 * H:]

    wbl = np.zeros((128, 1280), BF16)
    wbl[:, 0:128] = Wa1[:, IND:].T.astype(BF16)
    wbl[:, 128:256] = Wg1[:, IND:].T.astype(BF16)
    wbl[0:128, 256:320] = Wa2.T.astype(BF16)
    wbl[0:128, 320:384] = (-Wg2).T.astype(BF16)
    wbl[:, 384:512] = W_hh[0:H].T.astype(BF16)
    wbl[:, 512:640] = W_hh[H:2 * H].T.astype(BF16)
    wbl[:, 640:768] = W_hh[2 * H:].T.astype(BF16)
    wbl[0:65, 768:896] = W_ih[0:H].T.astype(BF16)
    wbl[0:65, 896:1024] = W_ih[H:2 * H].T.astype(BF16)
    wbl[0:65, 1024:1152] = W_ih[2 * H:].T.astype(BF16)
    wbl[0:1, 1152:1280] = b_hn[None, :].astype(BF16)

    bbl = np.zeros((128, 8), np.float32)
    bbl[:, 0] = ba1e
    bbl[:, 1] = bg1e
    bbl[0:64, 2] = b_out
    bbl[:, 3] = b_rz[:H]
    bbl[:, 4] = b_rz[H:]
    bbl[:, 5] = b_in

    in_maps = []
    for k in range(CORES):
        lo = c[k]
        sl = hiddens[lo:lo + S]
        if sl.shape[0] < S:
            sl = np.concatenate(
                [sl, np.zeros((S - sl.shape[0], H), np.float32)], axis=0)
        li = (ai[PC * k:PC * (k + 1)] - lo).astype(np.int64)
        idxw = np.zeros((128, NBLK * (BC // 16)), np.int16)
        for b in range(NBLK):
            vals = li[b * BC:(b + 1) * BC].copy()
            if b >= NBLK // 2:
                vals -= HALF
            idxw[:, b * (BC // 16):(b + 1) * (BC // 16)] = _wrap_idx(vals)
        in_maps.append({
            "hslf": np.ascontiguousarray(sl),
            "hslb": np.ascontiguousarray(sl.astype(BF16)),
            "idxw": idxw,
            "wbl": wbl,
            "bbl": bbl,
        })

    nc = _get_graph(step > 5)
    from concourse.bass_utils import run_bass_kernel_spmd
    res = run_bass_kernel_spmd(nc, in_maps, core_ids=list(range(CORES)))
    kernel._last_result = res
    kernel._last_in_maps = in_maps

    new_hiddens = np.empty((N_CELLS, H), np.float32)
    for k in range(CORES):
        new_hiddens[c[k]:c[k + 1]] = res.results[k]["outsl"][:c[k + 1] - c[k]]
    stats = res.results[0]["osmall"]
    wout = stats[0:64, 1]
    sum_e = stats[64, 2]
    sum_t = stats[64, 3]
    combined = (wout / sum_e).astype(np.float32)[None, :]
    mean_tension = np.float32(sum_t / NA)
    return combined, mean_tension, new_hiddens


kernel._last_result = None
kernel._last_in_maps = None


# revision 31
# speedup vs baseline: 1.2661x; 1.2661x over previous
"""Trainium2 Bass kernel for nn_AutopoieticEngine (scatter_memory).

Self-contained: takes FULL inputs (as produced by the problem's
setup_inputs), shards the cell dimension across 8 NeuronCores, runs a
Bass/Tile kernel per core (gather -> MLP+GRU -> faction sync/debate ->
scatter), all-reduces the tiny faction/softmax statistics on-device, and
reassembles the full outputs on the host.

Sharding: alive positions split into 8 chunks of 16384 = exactly one
faction per core (Na=131072, n_f=8, fs=16384).  Core k owns hiddens rows
[c_k, c_{k+1}) where c_k = alive_idx[16384*k], so its scatter targets are
entirely inside its own (padded) row slice.
"""

import numpy as np

try:
    import ml_dtypes

    BF16 = ml_dtypes.bfloat16
except Exception:  # pragma: no cover
    BF16 = None

CORES = 8
H = 128          # hidden dim
IND = 64         # input dim
D = 64           # out dim
N_CELLS = 262144
NA = 131072
PC = NA // CORES      # alive positions per core == faction size
S = 33792             # padded hiddens-slice rows per core (264*128)
HALF = 8192           # int16-index base split point (positions per core)
NBLK = 8              # cell blocks per core
BC = PC // NBLK       # 2048 cells per block
NCH = BC // 512       # 512-wide matmul chunks per block
CH = 512
SYNC = 0.15
DEBATE = 0.15
CPR = 4224            # copy-pass rows per tile (S/8, multiple of 128)

_CACHE = {}


def _variant():
    import os
    return frozenset(
        v for v in os.environ.get("KVAR", "").split(",") if v)


# --------------------------------------------------------------------------
# numpy fallback (exact reference semantics) for inputs that violate the
# layout assumptions this kernel hardcodes.
# --------------------------------------------------------------------------
def _np_reference(x, hiddens, Wa1, ba1, Wa2, ba2, Wg1, bg1, Wg2, bg2,
                  W_ih, W_hh, b_ih, b_hh, alive_idx, step):
    idx = np.asarray(alive_idx).astype(np.int64)
    h = hiddens[idx]
    xb = np.broadcast_to(x, (h.shape[0], x.shape[-1]))
    c = np.concatenate([xb, h], axis=-1)
    a = np.maximum(c @ Wa1.T + ba1, 0) @ Wa2.T + ba2
    g = np.maximum(c @ Wg1.T + bg1, 0) @ Wg2.T + bg2
    out = a - g
    tension = np.mean(out * out, axis=-1, keepdims=True)
    mem_in = np.concatenate([out, tension], axis=-1)
    gi = mem_in @ W_ih.T + b_ih
    gh = h @ W_hh.T + b_hh
    Hd = h.shape[-1]
    ir, iz, inn = gi[:, :Hd], gi[:, Hd:2 * Hd], gi[:, 2 * Hd:]
    hr, hz, hn = gh[:, :Hd], gh[:, Hd:2 * Hd], gh[:, 2 * Hd:]
    r = 1.0 / (1.0 + np.exp(-(ir + hr)))
    z = 1.0 / (1.0 + np.exp(-(iz + hz)))
    nn_ = np.tanh(inn + r * hn)
    new_h = (1.0 - z) * nn_ + z * h
    n, Hh = new_h.shape
    n_f = min(8, n // 2)
    if n_f >= 2:
        fs = n // n_f
        hb = new_h[: n_f * fs].reshape(n_f, fs, Hh)
        fm = hb.mean(axis=1, keepdims=True)
        hb = (1.0 - SYNC) * hb + SYNC * fm
        if step > 5:
            go = hb.mean(axis=1).mean(axis=0)
            dc = max(1, fs // 4)
            hb[:, :dc] = (1.0 - DEBATE) * hb[:, :dc] + DEBATE * go
        new_h = np.concatenate([hb.reshape(n_f * fs, Hh), new_h[n_f * fs:]], 0)
    new_hiddens = np.asarray(hiddens).copy()
    new_hiddens[idx] = new_h
    t = tension[:, 0]
    tm = t.max()
    w = np.exp(t - tm)
    w = w / w.sum()
    combined = (w[:, None] * out).sum(axis=0, keepdims=True)
    mean_tension = np.float32(t.mean())
    return (combined.astype(np.float32), mean_tension,
            new_hiddens.astype(np.float32))


# --------------------------------------------------------------------------
# graph builder
# --------------------------------------------------------------------------
def _build_graph(step_gt5: bool):
    import concourse.bass as bass
    import concourse.mybir as mybir
    import concourse.tile as tile
    from concourse import bacc
    from concourse.masks import make_identity

    var = _variant()

    f32 = mybir.dt.float32
    bf16 = mybir.dt.bfloat16
    i16 = mybir.dt.int16
    Alu = mybir.AluOpType
    Act = mybir.ActivationFunctionType

    nc = bacc.Bacc("TRN2", target_bir_lowering=False, debug=False,
                   num_devices=CORES)

    hslf = nc.declare_dram_parameter("hslf", [S, H], f32, isOutput=False)
    hslb = nc.declare_dram_parameter("hslb", [S, H], bf16, isOutput=False)
    idxw = nc.declare_dram_parameter("idxw", [128, NBLK * (BC // 16)], i16,
                                     isOutput=False)
    wbl = nc.declare_dram_parameter("wbl", [128, 1280], bf16, isOutput=False)
    bbl = nc.declare_dram_parameter("bbl", [128, 8], f32, isOutput=False)
    outsl = nc.declare_dram_parameter("outsl", [S, H], f32, isOutput=True)
    osmall = nc.declare_dram_parameter("osmall", [128, 4], f32, isOutput=True)

    arin = nc.dram_tensor("arin", [128, 4], f32)
    arout = nc.dram_tensor("arout", [128, 4], f32, addr_space="Shared")

    # constants for the fused sync/debate delta:
    #   delta = alpha*nn + beta*zd - d + v
    # non-debate: alpha=-SYNC, beta=1-SYNC, v=SYNC*fm
    # debate:     a2=(1-SYNC)(1-DEBATE); alpha=a2-1, beta=a2,
    #             v=SYNC*(1-DEBATE)*fm + DEBATE*go
    a2 = (1.0 - SYNC) * (1.0 - DEBATE)

    with tile.TileContext(nc) as tc:
        with (
            tc.tile_pool(name="const", bufs=1) as cpool,
            tc.tile_pool(name="per", bufs=1) as per,
            tc.tile_pool(name="ablk", bufs=1) as ablk,
            tc.tile_pool(name="h2", bufs=2) as h2,
            tc.tile_pool(name="bblk", bufs=2) as bblk,
            tc.tile_pool(name="cp", bufs=2) as cp,
            tc.tile_pool(name="ps", bufs=1, space="PSUM") as _ps0,
            tc.tile_pool(name="ps2", bufs=2, space="PSUM") as _ps2,
            tc.tile_pool(name="ps3", bufs=2, space="PSUM") as _ps3,
        ):
            HB = BC // 2  # 1024: half-block, one wide-psum tile
            # ---------------- constant / persistent tiles ----------------
            wsb = cpool.tile([128, 1280], bf16)
            bsb = cpool.tile([128, 8], f32)
            idxsb = cpool.tile([128, NBLK * (BC // 16)], i16)
            ident = cpool.tile([128, 128], bf16)
            ones_row = cpool.tile([1, CH], bf16)      # rhs for b_hn outer
            ones64 = cpool.tile([64, 65], bf16)       # tension lhsT (1/64)
            ones_t = cpool.tile([65, 64], bf16)       # e-bcast lhsT @ base 64

            nc.sync.dma_start(out=wsb[:], in_=wbl[:, :])
            nc.sync.dma_start(out=bsb[:], in_=bbl[:, :])
            nc.sync.dma_start(out=idxsb[:], in_=idxw[:, :])
            make_identity(nc, ident[:])
            nc.gpsimd.memset(ones_row[:], 1.0)
            nc.gpsimd.memset(ones64[:], 1.0 / 64.0)
            nc.gpsimd.memset(ones_t[:], 1.0)

            pa_cm = per.tile([128, PC], bf16)  # cell-major pa (transposed)
            mem = per.tile([65, PC], bf16)     # rows 0:64 out, row 64 t
            accn = per.tile([128, NBLK], f32)  # sum(nn) per block
            accz = per.tile([128, NBLK], f32)  # sum(z*d) per block
            acct = per.tile([65, NCH * NBLK], f32)  # row 64: sum(t) slots
            acce = per.tile([64, NCH * NBLK], f32)  # row0: sum(e) slots
            wacc = per.tile([64, NCH * NBLK], f32)  # wout partials
            sums = per.tile([128, 4], f32)     # AllReduce payload
            arsb = per.tile([128, 4], f32)     # AllReduce result
            vvec = per.tile([128, 2], f32)     # v (plain, debate)
            tmpv = per.tile([128, 1], f32)
            tmpn = per.tile([128, 1], f32)
            v_cm_p = per.tile([128, 128], bf16)   # v broadcast, plain
            v_cm_d = per.tile([128, 128], bf16)   # v broadcast, debate
            vcol_bf = per.tile([128, 1], bf16)
            vrow_bf = per.tile([1, 128], bf16)

            nc.gpsimd.memset(sums[:], 0.0)

            # ---------------- pass-through copy of the slice --------------
            for i in range(S // CPR):
                ct = cp.tile([128, CPR], f32, tag="cp")
                src = hslf[i * CPR:(i + 1) * CPR, :].rearrange(
                    "(p a) d -> p (a d)", p=128)
                dst = outsl[i * CPR:(i + 1) * CPR, :].rearrange(
                    "(p a) d -> p (a d)", p=128)
                nc.sync.dma_start(out=ct[:], in_=src)
                nc.sync.dma_start(out=dst, in_=ct[:])

            # ---------------- phase A ------------------------------------
            def emit_gather(b, h_tile):
                if "nogather" in var:
                    nc.gpsimd.memset(h_tile[:], 0.02)
                    return
                in_ap = hslb[:, :] if b < NBLK // 2 else hslb[HALF:, :]
                # transpose-mode dma_gather is limited to 512 idxs/call
                for c in range(NCH):
                    nc.gpsimd.dma_gather(
                        h_tile[:, c * CH:(c + 1) * CH].rearrange(
                            "p (a n) -> p a n", a=1),
                        in_ap,
                        idxsb[:, (b * NCH + c) * (CH // 16):
                              (b * NCH + c + 1) * (CH // 16)],
                        CH,
                        CH,
                        H,
                        transpose=True,
                    )

            h_tiles = {}
            h_tiles[0] = h2.tile([128, BC], bf16, tag="h", name="hbf0")
            emit_gather(0, h_tiles[0])

            for b in range(NBLK):
                blk = slice(b * BC, (b + 1) * BC)
                h_bf = h_tiles.pop(b)
                if b + 1 < NBLK:
                    h_tiles[b + 1] = h2.tile([128, BC], bf16, tag="h", name=f"hbf{b+1}")
                    emit_gather(b + 1, h_tiles[b + 1])

                a1 = ablk.tile([128, BC], bf16, tag="a1")
                g1 = ablk.tile([128, BC], bf16, tag="g1")
                sq = ablk.tile([64, BC], bf16, tag="sq")
                r_bf = ablk.tile([128, BC], bf16, tag="r")
                z_bf = ablk.tile([128, BC], bf16, tag="z")
                rhn = ablk.tile([128, BC], bf16, tag="rhn")
                s_bf = ablk.tile([128, BC], bf16, tag="s")
                nn_bf = ablk.tile([128, BC], bf16, tag="nn")
                d_bf = ablk.tile([128, BC], bf16, tag="d")
                zd_bf = ablk.tile([128, BC], bf16, tag="zd")
                q1 = ablk.tile([128, BC], bf16, tag="q1")

                # stage 1/2: first-layer MLPs (x-part folded into bias)
                for c in range(NCH):
                    cs = slice(c * CH, (c + 1) * CH)
                    p1 = _ps2.tile([128, CH], f32, tag="p1")
                    nc.tensor.matmul(out=p1[:], lhsT=wsb[:, 0:128],
                                     rhs=h_bf[:, cs], start=True, stop=True)
                    nc.scalar.activation(a1[:, cs], p1[:], Act.Relu,
                                         bias=bsb[:, 0:1])
                for c in range(NCH):
                    cs = slice(c * CH, (c + 1) * CH)
                    p1 = _ps2.tile([128, CH], f32, tag="p1")
                    nc.tensor.matmul(out=p1[:], lhsT=wsb[:, 128:256],
                                     rhs=h_bf[:, cs], start=True, stop=True)
                    nc.scalar.activation(g1[:, cs], p1[:], Act.Relu,
                                         bias=bsb[:, 1:2])

                # stage 3: out = Wa2@a1 - Wg2@g1 + b_out  -> mem rows 0:64
                for c in range(NCH):
                    cs = slice(c * CH, (c + 1) * CH)
                    mcs = slice(b * BC + c * CH, b * BC + (c + 1) * CH)
                    po = _ps2.tile([64, CH], f32, tag="p1")
                    nc.tensor.matmul(out=po[:], lhsT=wsb[0:128, 256:320],
                                     rhs=a1[:, cs], start=True, stop=False)
                    nc.tensor.matmul(out=po[:], lhsT=wsb[0:128, 320:384],
                                     rhs=g1[:, cs], start=False, stop=True)
                    nc.scalar.activation(mem[0:64, mcs], po[:], Act.Identity,
                                         bias=bsb[0:64, 2:3])

                # tension: t = mean(out^2) over features -> mem row 64
                nc.vector.tensor_tensor(out=sq[:], in0=mem[0:64, blk],
                                        in1=mem[0:64, blk], op=Alu.mult)
                for c in range(NCH):
                    cs = slice(c * CH, (c + 1) * CH)
                    mcs = slice(b * BC + c * CH, b * BC + (c + 1) * CH)
                    pt = _ps2.tile([65, CH], f32, tag="p1")
                    nc.tensor.matmul(out=pt[:, :], lhsT=ones64[:, :],
                                     rhs=sq[:, cs], start=True, stop=True)
                    nc.scalar.activation(
                        mem[64:65, mcs], pt[64:65, :], Act.Copy,
                        accum_out=acct[64:65, b * NCH + c:b * NCH + c + 1])

                # GRU gates
                for c in range(NCH):
                    cs = slice(c * CH, (c + 1) * CH)
                    mcs = slice(b * BC + c * CH, b * BC + (c + 1) * CH)
                    pr = _ps0.tile([128, CH], f32, tag="pr")
                    pz = _ps0.tile([128, CH], f32, tag="pz")
                    pi = _ps0.tile([128, CH], f32, tag="pi")
                    ph = _ps0.tile([128, CH], f32, tag="ph")
                    nc.tensor.matmul(out=pr[:], lhsT=wsb[0:65, 768:896],
                                     rhs=mem[0:65, mcs], start=True,
                                     stop=False)
                    nc.tensor.matmul(out=pr[:], lhsT=wsb[:, 384:512],
                                     rhs=h_bf[:, cs], start=False, stop=True)
                    nc.tensor.matmul(out=pz[:], lhsT=wsb[0:65, 896:1024],
                                     rhs=mem[0:65, mcs], start=True,
                                     stop=False)
                    nc.tensor.matmul(out=pz[:], lhsT=wsb[:, 512:640],
                                     rhs=h_bf[:, cs], start=False, stop=True)
                    nc.tensor.matmul(out=pi[:], lhsT=wsb[0:65, 1024:1152],
                                     rhs=mem[0:65, mcs], start=True,
                                     stop=True)
                    nc.tensor.matmul(out=ph[:], lhsT=wsb[0:1, 1152:1280],
                                     rhs=ones_row[:, :], start=True,
                                     stop=False)
                    nc.tensor.matmul(out=ph[:], lhsT=wsb[:, 640:768],
                                     rhs=h_bf[:, cs], start=False, stop=True)
                    nc.scalar.activation(r_bf[:, cs], pr[:], Act.Sigmoid,
                                         bias=bsb[:, 3:4])
                    nc.scalar.activation(z_bf[:, cs], pz[:], Act.Sigmoid,
                                         bias=bsb[:, 4:5])
                    nc.vector.tensor_tensor(out=rhn[:, cs], in0=r_bf[:, cs],
                                            in1=ph[:], op=Alu.mult)
                    nc.vector.tensor_tensor(out=s_bf[:, cs], in0=rhn[:, cs],
                                            in1=pi[:], op=Alu.add)

                nc.scalar.activation(nn_bf[:], s_bf[:], Act.Tanh,
                                     bias=bsb[:, 5:6],
                                     accum_out=accn[:, b:b + 1])
                nc.vector.tensor_tensor(out=d_bf[:], in0=h_bf[:],
                                        in1=nn_bf[:], op=Alu.subtract)
                nc.vector.scalar_tensor_tensor(
                    out=zd_bf[:], in0=z_bf[:], scalar=1.0, in1=d_bf[:],
                    op0=Alu.mult, op1=Alu.mult,
                    accum_out=accz[:, b:b + 1])
                deb = step_gt5 and (b * BC < PC // 4)
                alpha = (a2 - 1.0) if deb else -SYNC
                beta = a2 if deb else (1.0 - SYNC)
                nc.vector.scalar_tensor_tensor(
                    out=q1[:], in0=zd_bf[:], scalar=beta, in1=d_bf[:],
                    op0=Alu.mult, op1=Alu.subtract)
                pa_b = ablk.tile([128, BC], bf16, tag="pab")
                nc.vector.scalar_tensor_tensor(
                    out=pa_b[:], in0=nn_bf[:], scalar=alpha, in1=q1[:],
                    op0=Alu.mult, op1=Alu.add)
                # transpose to cell-major now; v is added later in phase B
                for hh in range(2):
                    ptr = _ps3.tile([128, HB], bf16, tag="ptr")
                    for j in range(HB // 128):
                        col = hh * HB + j * 128
                        nc.tensor.transpose(
                            out=ptr[:, j * 128:(j + 1) * 128],
                            in_=pa_b[:, col:col + 128],
                            identity=ident[:])
                    dsl = slice(b * BC + hh * HB, b * BC + (hh + 1) * HB)
                    if (b + hh) % 2 == 0:
                        nc.scalar.activation(pa_cm[:, dsl], ptr[:], Act.Copy)
                    else:
                        nc.vector.tensor_copy(pa_cm[:, dsl], ptr[:])

            # ---------------- local stats (no AllReduce needed) -----------
            nc.vector.tensor_reduce(out=tmpn[:], in_=accn[:, :],
                                    axis=mybir.AxisListType.X, op=Alu.add)
            nc.vector.tensor_reduce(out=tmpv[:], in_=accz[:, :],
                                    axis=mybir.AxisListType.X, op=Alu.add)
            nc.vector.tensor_tensor(out=sums[:, 0:1], in0=tmpn[:],
                                    in1=tmpv[:], op=Alu.add)
            # v_plain = SYNC/PC * local_sum
            nc.vector.tensor_scalar(out=vvec[:, 0:1], in0=sums[:, 0:1],
                                    scalar1=SYNC / PC, scalar2=None,
                                    op0=Alu.mult)

            def emit_vcm(col, dst):
                # dst[i, j] = vvec[j, col] for all i (broadcast matrix)
                nc.vector.tensor_copy(vcol_bf[:], vvec[:, col:col + 1])
                pv1 = _ps0.tile([1, 128], f32, tag="pz", name=f"pv1_{col}")
                nc.tensor.matmul(out=pv1[:], lhsT=vcol_bf[:],
                                 rhs=ident[:], start=True, stop=True)
                nc.scalar.activation(vrow_bf[:], pv1[:], Act.Copy)
                pv2 = _ps0.tile([128, 128], f32, tag="pr", name=f"pv2_{col}")
                nc.tensor.matmul(out=pv2[:], lhsT=ones_row[0:1, 0:128],
                                 rhs=vrow_bf[:], start=True, stop=True)
                nc.scalar.activation(dst[:], pv2[:], Act.Copy)

            emit_vcm(0, v_cm_p)

            # ---------------- phase B ------------------------------------
            def emit_phase_b(b):
                deb = step_gt5 and (b * BC < PC // 4)
                vt = v_cm_d if deb else v_cm_p
                dcm = bblk.tile([128, BC], f32, tag="dcm")
                nc.vector.tensor_tensor(
                    out=dcm[:].rearrange("p (a n) -> p a n", n=H),
                    in0=pa_cm[:, b * BC:(b + 1) * BC].rearrange(
                        "p (a n) -> p a n", n=H),
                    in1=vt[:, None, :].to_broadcast([128, BC // H, H]),
                    op=Alu.add)
                if "noscatter" not in var:
                    out_ap = outsl[:, :] if b < NBLK // 2 \
                        else outsl[HALF:, :]
                    idx_ap = idxsb[:, b * (BC // 16):(b + 1) * (BC // 16)]
                    nc.gpsimd.dma_scatter_add(
                        out_ap,
                        dcm[:].rearrange("p (a n) -> p a n", n=H),
                        idx_ap,
                        BC,
                        BC,
                        H,
                    )

            ndeb = (PC // 4) // BC
            for b in range(ndeb, NBLK):
                emit_phase_b(b)

            # ---------------- global stats + AllReduce --------------------
            nc.vector.tensor_reduce(out=sums[64:65, 3:4],
                                    in_=acct[64:65, :],
                                    axis=mybir.AxisListType.X, op=Alu.add)
            # wout: bcast raw t via PE, exp on ACT (exp(bcast)=bcast(exp)),
            # multiply-accumulate on DVE at 2x (both operands SBUF bf16)
            for c in range(NCH * NBLK):
                cs = slice(c * CH, (c + 1) * CH)
                pe = _ps0.tile([64, CH], f32, tag="pz")
                nc.tensor.matmul(out=pe[:], lhsT=ones_t[64:65, 0:64],
                                 rhs=mem[64:65, cs], start=True, stop=True)
                e_sb = bblk.tile([64, CH], bf16, tag="esb")
                nc.scalar.activation(e_sb[:], pe[:], Act.Exp,
                                     accum_out=acce[0:64, c:c + 1])
                eo = bblk.tile([64, CH], bf16, tag="eo")
                nc.vector.scalar_tensor_tensor(
                    out=eo[:], in0=mem[0:64, cs], scalar=1.0, in1=e_sb[:],
                    op0=Alu.mult, op1=Alu.mult,
                    accum_out=wacc[:, c:c + 1])
            nc.vector.tensor_reduce(out=sums[0:64, 1:2], in_=wacc[:, :],
                                    axis=mybir.AxisListType.X, op=Alu.add)
            nc.vector.tensor_reduce(out=sums[0:1, 2:3], in_=acce[0:1, :],
                                    axis=mybir.AxisListType.X, op=Alu.add)

            if "nocoll" in var:
                nc.vector.tensor_scalar(out=arsb[:], in0=sums[:],
                                        scalar1=float(CORES), scalar2=None,
                                        op0=Alu.mult)
            else:
                nc.gpsimd.dma_start(out=arin[:, :], in_=sums[:])
                nc.gpsimd.collective_compute(
                    "AllReduce", Alu.add,
                    replica_groups=[list(range(CORES))],
                    ins=[arin.ap().opt()],
                    outs=[arout.ap().opt()],
                )
                nc.gpsimd.dma_start(out=arsb[:], in_=arout[:, :])
            nc.sync.dma_start(out=osmall[:, :], in_=arsb[:])

            # v_debate = SYNC*(1-DEBATE)/PC * local + DEBATE/NA * total
            nc.vector.tensor_scalar(out=tmpv[:], in0=sums[:, 0:1],
                                    scalar1=SYNC * (1.0 - DEBATE) / PC,
                                    scalar2=None, op0=Alu.mult)
            nc.vector.scalar_tensor_tensor(
                out=vvec[:, 1:2], in0=arsb[:, 0:1], scalar=DEBATE / NA,
                in1=tmpv[:], op0=Alu.mult, op1=Alu.add)
            emit_vcm(1, v_cm_d)

            for b in range(ndeb):
                emit_phase_b(b)

    nc.compile()
    return nc


def _get_graph(step_gt5: bool):
    key = (bool(step_gt5), _variant())
    if key not in _CACHE:
        _CACHE[key] = _build_graph(step_gt5)
    return _CACHE[key]


# --------------------------------------------------------------------------
# host-side sharding + launch
# --------------------------------------------------------------------------
def _wrap_idx(vals):
    """int16 index layout for dma_gather/dma_scatter_add: [128, n//16],
    idx q stored at [q % 16, q // 16], replicated to all 8 Q7 groups."""
    n = vals.shape[0]
    w = vals.reshape(n // 16, 16).T.astype(np.int16)    # [16, n//16]
    return np.tile(w, (8, 1))                            # [128, n//16]


def kernel(**inputs):
    x = np.asarray(inputs["x"], np.float32)
    hiddens = np.asarray(inputs["hiddens"], np.float32)
    Wa1 = np.asarray(inputs["Wa1"], np.float32)
    ba1 = np.asarray(inputs["ba1"], np.float32)
    Wa2 = np.asarray(inputs["Wa2"], np.float32)
    ba2 = np.asarray(inputs["ba2"], np.float32)
    Wg1 = np.asarray(inputs["Wg1"], np.float32)
    bg1 = np.asarray(inputs["bg1"], np.float32)
    Wg2 = np.asarray(inputs["Wg2"], np.float32)
    bg2 = np.asarray(inputs["bg2"], np.float32)
    W_ih = np.asarray(inputs["W_ih"], np.float32)
    W_hh = np.asarray(inputs["W_hh"], np.float32)
    b_ih = np.asarray(inputs["b_ih"], np.float32)
    b_hh = np.asarray(inputs["b_hh"], np.float32)
    ai = np.asarray(inputs["alive_idx"]).astype(np.int64)
    step = int(inputs["step"])

    ok = (
        BF16 is not None
        and hiddens.shape == (N_CELLS, H)
        and ai.shape == (NA,)
        and np.all(np.diff(ai) > 0)
        and step > 5
    )
    if ok:
        c = [0] + [int(ai[PC * k]) for k in range(1, CORES)] + [N_CELLS]
        lens = [c[k + 1] - c[k] for k in range(CORES)]
        for k in range(CORES):
            li = ai[PC * k:PC * (k + 1)] - c[k]
            if lens[k] > S or li[:HALF].max() > 32000 or \
               li[HALF:].min() < HALF or (li[HALF:] - HALF).max() > S - HALF - 1:
                ok = False
                break
    if not ok:
        return _np_reference(x, hiddens, Wa1, ba1, Wa2, ba2, Wg1, bg1, Wg2,
                             bg2, W_ih, W_hh, b_ih, b_hh, ai, step)

    # ---- replicated weight prep (x-part of layer-1 folded into biases) ----
    ba1e = ba1 + Wa1[:, :IND] @ x[0]
    bg1e = bg1 + Wg1[:, :IND] @ x[0]
    b_out = ba2 - bg2
    b_rz = (b_ih + b_hh)[:2 * H]
    b_in = b_ih[2 * H:]
    b_hn = b_hh[2 * H:]

    wbl = np.zeros((128, 1280), BF16)
    wbl[:, 0:128] = Wa1[:, IND:].T.astype(BF16)
    wbl[:, 128:256] = Wg1[:, IND:].T.astype(BF16)
    wbl[0:128, 256:320] = Wa2.T.astype(BF16)
    wbl[0:128, 320:384] = (-Wg2).T.astype(BF16)
    wbl[:, 384:512] = W_hh[0:H].T.astype(BF16)
    wbl[:, 512:640] = W_hh[H:2 * H].T.astype(BF16)
    wbl[:, 640:768] = W_hh[2 * H:].T.astype(BF16)
    wbl[0:65, 768:896] = W_ih[0:H].T.astype(BF16)
    wbl[0:65, 896:1024] = W_ih[H:2 * H].T.astype(BF16)
    wbl[0:65, 1024:1152] = W_ih[2 * H:].T.astype(BF16)
    wbl[0:1, 1152:1280] = b_hn[None, :].astype(BF16)

    bbl = np.zeros((128, 8), np.float32)
    bbl[:, 0] = ba1e
    bbl[:, 1] = bg1e
    bbl[0:64, 2] = b_out
    bbl[:, 3] = b_rz[:H]
    bbl[:, 4] = b_rz[H:]
    bbl[:, 5] = b_in

    in_maps = []
    for k in range(CORES):
        lo = c[k]
        sl = hiddens[lo:lo + S]
        if sl.shape[0] < S:
            sl = np.concatenate(
                [sl, np.zeros((S - sl.shape[0], H), np.float32)], axis=0)
        li = (ai[PC * k:PC * (k + 1)] - lo).astype(np.int64)
        idxw = np.zeros((128, NBLK * (BC // 16)), np.int16)
        for b in range(NBLK):
            vals = li[b * BC:(b + 1) * BC].copy()
            if b >= NBLK // 2:
                vals -= HALF
            idxw[:, b * (BC // 16):(b + 1) * (BC // 16)] = _wrap_idx(vals)
        in_maps.append({
            "hslf": np.ascontiguousarray(sl),
            "hslb": np.ascontiguousarray(sl.astype(BF16)),
            "idxw": idxw,
            "wbl": wbl,
            "bbl": bbl,
        })

    nc = _get_graph(step > 5)
    from concourse.bass_utils import run_bass_kernel_spmd
    res = run_bass_kernel_spmd(nc, in_maps, core_ids=list(range(CORES)))
    kernel._last_result = res
    kernel._last_in_maps = in_maps

    new_hiddens = np.empty((N_CELLS, H), np.float32)
    for k in range(CORES):
        new_hiddens[c[k]:c[k + 1]] = res.results[k]["outsl"][:c[k + 1] - c[k]]
    stats = res.results[0]["osmall"]
    wout = stats[0:64, 1]
    sum_e = stats[0, 2]
    sum_t = stats[64, 3]
    combined = (wout / sum_e).astype(np.float32)[None, :]
    mean_tension = np.float32(sum_t / NA)
    return combined, mean_tension, new_hiddens


kernel._last_result = None
kernel._last_in_maps = None


# revision 32
# speedup vs baseline: 1.2739x; 1.0061x over previous
"""Trainium2 Bass kernel for nn_AutopoieticEngine (scatter_memory).

Self-contained: takes FULL inputs (as produced by the problem's
setup_inputs), shards the cell dimension across 8 NeuronCores, runs a
Bass/Tile kernel per core (gather -> MLP+GRU -> faction sync/debate ->
scatter), all-reduces the tiny faction/softmax statistics on-device, and
reassembles the full outputs on the host.

Sharding: alive positions split into 8 chunks of 16384 = exactly one
faction per core (Na=131072, n_f=8, fs=16384).  Core k owns hiddens rows
[c_k, c_{k+1}) where c_k = alive_idx[16384*k], so its scatter targets are
entirely inside its own (padded) row slice.
"""

import numpy as np

try:
    import ml_dtypes

    BF16 = ml_dtypes.bfloat16
except Exception:  # pragma: no cover
    BF16 = None

CORES = 8
H = 128          # hidden dim
IND = 64         # input dim
D = 64           # out dim
N_CELLS = 262144
NA = 131072
PC = NA // CORES      # alive positions per core == faction size
S = 33792             # padded hiddens-slice rows per core (264*128)
HALF = 8192           # int16-index base split point (positions per core)
NBLK = 8              # cell blocks per core
BC = PC // NBLK       # 2048 cells per block
NCH = BC // 512       # 512-wide matmul chunks per block
CH = 512
SYNC = 0.15
DEBATE = 0.15
CPR = 4224            # copy-pass rows per tile (S/8, multiple of 128)

_CACHE = {}


def _variant():
    import os
    return frozenset(
        v for v in os.environ.get("KVAR", "").split(",") if v)


# --------------------------------------------------------------------------
# numpy fallback (exact reference semantics) for inputs that violate the
# layout assumptions this kernel hardcodes.
# --------------------------------------------------------------------------
def _np_reference(x, hiddens, Wa1, ba1, Wa2, ba2, Wg1, bg1, Wg2, bg2,
                  W_ih, W_hh, b_ih, b_hh, alive_idx, step):
    idx = np.asarray(alive_idx).astype(np.int64)
    h = hiddens[idx]
    xb = np.broadcast_to(x, (h.shape[0], x.shape[-1]))
    c = np.concatenate([xb, h], axis=-1)
    a = np.maximum(c @ Wa1.T + ba1, 0) @ Wa2.T + ba2
    g = np.maximum(c @ Wg1.T + bg1, 0) @ Wg2.T + bg2
    out = a - g
    tension = np.mean(out * out, axis=-1, keepdims=True)
    mem_in = np.concatenate([out, tension], axis=-1)
    gi = mem_in @ W_ih.T + b_ih
    gh = h @ W_hh.T + b_hh
    Hd = h.shape[-1]
    ir, iz, inn = gi[:, :Hd], gi[:, Hd:2 * Hd], gi[:, 2 * Hd:]
    hr, hz, hn = gh[:, :Hd], gh[:, Hd:2 * Hd], gh[:, 2 * Hd:]
    r = 1.0 / (1.0 + np.exp(-(ir + hr)))
    z = 1.0 / (1.0 + np.exp(-(iz + hz)))
    nn_ = np.tanh(inn + r * hn)
    new_h = (1.0 - z) * nn_ + z * h
    n, Hh = new_h.shape
    n_f = min(8, n // 2)
    if n_f >= 2:
        fs = n // n_f
        hb = new_h[: n_f * fs].reshape(n_f, fs, Hh)
        fm = hb.mean(axis=1, keepdims=True)
        hb = (1.0 - SYNC) * hb + SYNC * fm
        if step > 5:
            go = hb.mean(axis=1).mean(axis=0)
            dc = max(1, fs // 4)
            hb[:, :dc] = (1.0 - DEBATE) * hb[:, :dc] + DEBATE * go
        new_h = np.concatenate([hb.reshape(n_f * fs, Hh), new_h[n_f * fs:]], 0)
    new_hiddens = np.asarray(hiddens).copy()
    new_hiddens[idx] = new_h
    t = tension[:, 0]
    tm = t.max()
    w = np.exp(t - tm)
    w = w / w.sum()
    combined = (w[:, None] * out).sum(axis=0, keepdims=True)
    mean_tension = np.float32(t.mean())
    return (combined.astype(np.float32), mean_tension,
            new_hiddens.astype(np.float32))


# --------------------------------------------------------------------------
# graph builder
# --------------------------------------------------------------------------
def _build_graph(step_gt5: bool):
    import concourse.bass as bass
    import concourse.mybir as mybir
    import concourse.tile as tile
    from concourse import bacc
    from concourse.masks import make_identity

    var = _variant()

    f32 = mybir.dt.float32
    bf16 = mybir.dt.bfloat16
    i16 = mybir.dt.int16
    Alu = mybir.AluOpType
    Act = mybir.ActivationFunctionType

    nc = bacc.Bacc("TRN2", target_bir_lowering=False, debug=False,
                   num_devices=CORES)

    hslf = nc.declare_dram_parameter("hslf", [S, H], f32, isOutput=False)
    hslb = nc.declare_dram_parameter("hslb", [S, H], bf16, isOutput=False)
    idxw = nc.declare_dram_parameter("idxw", [128, NBLK * (BC // 16)], i16,
                                     isOutput=False)
    wbl = nc.declare_dram_parameter("wbl", [128, 1280], bf16, isOutput=False)
    bbl = nc.declare_dram_parameter("bbl", [128, 8], f32, isOutput=False)
    outsl = nc.declare_dram_parameter("outsl", [S, H], f32, isOutput=True)
    osmall = nc.declare_dram_parameter("osmall", [128, 4], f32, isOutput=True)

    arin = nc.dram_tensor("arin", [128, 4], f32)
    arout = nc.dram_tensor("arout", [128, 4], f32, addr_space="Shared")

    # constants for the fused sync/debate delta:
    #   delta = alpha*nn + beta*zd - d + v
    # non-debate: alpha=-SYNC, beta=1-SYNC, v=SYNC*fm
    # debate:     a2=(1-SYNC)(1-DEBATE); alpha=a2-1, beta=a2,
    #             v=SYNC*(1-DEBATE)*fm + DEBATE*go
    a2 = (1.0 - SYNC) * (1.0 - DEBATE)

    with tile.TileContext(nc) as tc:
        with (
            tc.tile_pool(name="const", bufs=1) as cpool,
            tc.tile_pool(name="per", bufs=1) as per,
            tc.tile_pool(name="ablk", bufs=1) as ablk,
            tc.tile_pool(name="h2", bufs=2) as h2,
            tc.tile_pool(name="bblk", bufs=2) as bblk,
            tc.tile_pool(name="cp", bufs=2) as cp,
            tc.tile_pool(name="ps", bufs=1, space="PSUM") as _ps0,
            tc.tile_pool(name="ps2", bufs=2, space="PSUM") as _ps2,
            tc.tile_pool(name="ps3", bufs=2, space="PSUM") as _ps3,
        ):
            HB = BC // 2  # 1024: half-block, one wide-psum tile
            # ---------------- constant / persistent tiles ----------------
            wsb = cpool.tile([128, 1280], bf16)
            bsb = cpool.tile([128, 8], f32)
            idxsb = cpool.tile([128, NBLK * (BC // 16)], i16)
            ident = cpool.tile([128, 128], bf16)
            ones_row = cpool.tile([1, CH], bf16)      # rhs for b_hn outer
            ones64 = cpool.tile([64, 65], bf16)       # tension lhsT (1/64)
            ones_t = cpool.tile([65, 64], bf16)       # e-bcast lhsT @ base 64

            nc.sync.dma_start(out=wsb[:], in_=wbl[:, :])
            nc.sync.dma_start(out=bsb[:], in_=bbl[:, :])
            nc.sync.dma_start(out=idxsb[:], in_=idxw[:, :])
            make_identity(nc, ident[:])
            nc.gpsimd.memset(ones_row[:], 1.0)
            nc.gpsimd.memset(ones64[:], 1.0 / 64.0)
            nc.gpsimd.memset(ones_t[:], 1.0)

            pa_cm = per.tile([128, PC], bf16)  # cell-major pa (transposed)
            mem = per.tile([65, PC], bf16)     # rows 0:64 out, row 64 t
            accn = per.tile([128, NBLK], f32)  # sum(nn) per block
            accz = per.tile([128, NBLK], f32)  # sum(z*d) per block
            acct = per.tile([65, NCH * NBLK], f32)  # row 64: sum(t) slots
            acce = per.tile([64, NCH * NBLK], f32)  # row0: sum(e) slots
            wacc = per.tile([64, NCH * NBLK], f32)  # wout partials
            sums = per.tile([128, 4], f32)     # AllReduce payload
            arsb = per.tile([128, 4], f32)     # AllReduce result
            vvec = per.tile([128, 2], f32)     # v (plain, debate)
            tmpv = per.tile([128, 1], f32)
            tmpn = per.tile([128, 1], f32)
            v_cm_p = per.tile([128, 128], bf16)   # v broadcast, plain
            v_cm_d = per.tile([128, 128], bf16)   # v broadcast, debate
            vcol_bf = per.tile([128, 1], bf16)
            vrow_bf = per.tile([1, 128], bf16)

            nc.gpsimd.memset(sums[:], 0.0)

            # ---------------- pass-through copy of the slice --------------
            for i in range(S // CPR):
                ct = cp.tile([128, CPR], f32, tag="cp")
                src = hslf[i * CPR:(i + 1) * CPR, :].rearrange(
                    "(p a) d -> p (a d)", p=128)
                dst = outsl[i * CPR:(i + 1) * CPR, :].rearrange(
                    "(p a) d -> p (a d)", p=128)
                nc.sync.dma_start(out=ct[:], in_=src)
                nc.sync.dma_start(out=dst, in_=ct[:])

            # ---------------- phase A ------------------------------------
            def emit_gather(b, h_tile):
                if "nogather" in var:
                    nc.gpsimd.memset(h_tile[:], 0.02)
                    return
                in_ap = hslb[:, :] if b < NBLK // 2 else hslb[HALF:, :]
                # transpose-mode dma_gather is limited to 512 idxs/call
                for c in range(NCH):
                    nc.gpsimd.dma_gather(
                        h_tile[:, c * CH:(c + 1) * CH].rearrange(
                            "p (a n) -> p a n", a=1),
                        in_ap,
                        idxsb[:, (b * NCH + c) * (CH // 16):
                              (b * NCH + c + 1) * (CH // 16)],
                        CH,
                        CH,
                        H,
                        transpose=True,
                    )

            h_tiles = {}
            h_tiles[0] = h2.tile([128, BC], bf16, tag="h", name="hbf0")
            emit_gather(0, h_tiles[0])

            for b in range(NBLK):
                blk = slice(b * BC, (b + 1) * BC)
                h_bf = h_tiles.pop(b)
                if b + 1 < NBLK:
                    h_tiles[b + 1] = h2.tile([128, BC], bf16, tag="h", name=f"hbf{b+1}")
                    emit_gather(b + 1, h_tiles[b + 1])

                a1 = ablk.tile([128, BC], bf16, tag="a1")
                g1 = ablk.tile([128, BC], bf16, tag="g1")
                sq = ablk.tile([64, BC], bf16, tag="sq")
                r_bf = ablk.tile([128, BC], bf16, tag="r")
                z_bf = ablk.tile([128, BC], bf16, tag="z")
                rhn = ablk.tile([128, BC], bf16, tag="rhn")
                s_bf = ablk.tile([128, BC], bf16, tag="s")
                nn_bf = ablk.tile([128, BC], bf16, tag="nn")
                d_bf = ablk.tile([128, BC], bf16, tag="d")
                zd_bf = ablk.tile([128, BC], bf16, tag="zd")
                q1 = ablk.tile([128, BC], bf16, tag="q1")

                # stage 1/2: first-layer MLPs (x-part folded into bias)
                for c in range(NCH):
                    cs = slice(c * CH, (c + 1) * CH)
                    p1 = _ps2.tile([128, CH], f32, tag="p1")
                    nc.tensor.matmul(out=p1[:], lhsT=wsb[:, 0:128],
                                     rhs=h_bf[:, cs], start=True, stop=True)
                    nc.scalar.activation(a1[:, cs], p1[:], Act.Relu,
                                         bias=bsb[:, 0:1])
                for c in range(NCH):
                    cs = slice(c * CH, (c + 1) * CH)
                    p1 = _ps2.tile([128, CH], f32, tag="p1")
                    nc.tensor.matmul(out=p1[:], lhsT=wsb[:, 128:256],
                                     rhs=h_bf[:, cs], start=True, stop=True)
                    nc.scalar.activation(g1[:, cs], p1[:], Act.Relu,
                                         bias=bsb[:, 1:2])

                # stage 3: out = Wa2@a1 - Wg2@g1 + b_out  -> mem rows 0:64
                for c in range(NCH):
                    cs = slice(c * CH, (c + 1) * CH)
                    mcs = slice(b * BC + c * CH, b * BC + (c + 1) * CH)
                    po = _ps2.tile([64, CH], f32, tag="p1")
                    nc.tensor.matmul(out=po[:], lhsT=wsb[0:128, 256:320],
                                     rhs=a1[:, cs], start=True, stop=False)
                    nc.tensor.matmul(out=po[:], lhsT=wsb[0:128, 320:384],
                                     rhs=g1[:, cs], start=False, stop=True)
                    nc.scalar.activation(mem[0:64, mcs], po[:], Act.Identity,
                                         bias=bsb[0:64, 2:3])

                # tension: t = mean(out^2) over features -> mem row 64
                nc.vector.tensor_tensor(out=sq[:], in0=mem[0:64, blk],
                                        in1=mem[0:64, blk], op=Alu.mult)
                for c in range(NCH):
                    cs = slice(c * CH, (c + 1) * CH)
                    mcs = slice(b * BC + c * CH, b * BC + (c + 1) * CH)
                    pt = _ps2.tile([65, CH], f32, tag="p1")
                    nc.tensor.matmul(out=pt[:, :], lhsT=ones64[:, :],
                                     rhs=sq[:, cs], start=True, stop=True)
                    nc.scalar.activation(
                        mem[64:65, mcs], pt[64:65, :], Act.Copy,
                        accum_out=acct[64:65, b * NCH + c:b * NCH + c + 1])

                # GRU gates
                for c in range(NCH):
                    cs = slice(c * CH, (c + 1) * CH)
                    mcs = slice(b * BC + c * CH, b * BC + (c + 1) * CH)
                    pr = _ps0.tile([128, CH], f32, tag="pr")
                    pz = _ps0.tile([128, CH], f32, tag="pz")
                    pi = _ps0.tile([128, CH], f32, tag="pi")
                    ph = _ps0.tile([128, CH], f32, tag="ph")
                    nc.tensor.matmul(out=pr[:], lhsT=wsb[0:65, 768:896],
                                     rhs=mem[0:65, mcs], start=True,
                                     stop=False)
                    nc.tensor.matmul(out=pr[:], lhsT=wsb[:, 384:512],
                                     rhs=h_bf[:, cs], start=False, stop=True)
                    nc.tensor.matmul(out=pz[:], lhsT=wsb[0:65, 896:1024],
                                     rhs=mem[0:65, mcs], start=True,
                                     stop=False)
                    nc.tensor.matmul(out=pz[:], lhsT=wsb[:, 512:640],
                                     rhs=h_bf[:, cs], start=False, stop=True)
                    nc.tensor.matmul(out=pi[:], lhsT=wsb[0:65, 1024:1152],
                                     rhs=mem[0:65, mcs], start=True,
                                     stop=True)
                    nc.tensor.matmul(out=ph[:], lhsT=wsb[0:1, 1152:1280],
                                     rhs=ones_row[:, :], start=True,
                                     stop=False)
                    nc.tensor.matmul(out=ph[:], lhsT=wsb[:, 640:768],
                                     rhs=h_bf[:, cs], start=False, stop=True)
                    nc.scalar.activation(r_bf[:, cs], pr[:], Act.Sigmoid,
                                         bias=bsb[:, 3:4])
                    nc.scalar.activation(z_bf[:, cs], pz[:], Act.Sigmoid,
                                         bias=bsb[:, 4:5])
                    nc.vector.tensor_tensor(out=rhn[:, cs], in0=r_bf[:, cs],
                                            in1=ph[:], op=Alu.mult)
                    nc.vector.tensor_tensor(out=s_bf[:, cs], in0=rhn[:, cs],
                                            in1=pi[:], op=Alu.add)

                nc.scalar.activation(nn_bf[:], s_bf[:], Act.Tanh,
                                     bias=bsb[:, 5:6],
                                     accum_out=accn[:, b:b + 1])
                nc.vector.tensor_tensor(out=d_bf[:], in0=h_bf[:],
                                        in1=nn_bf[:], op=Alu.subtract)
                nc.vector.scalar_tensor_tensor(
                    out=zd_bf[:], in0=z_bf[:], scalar=1.0, in1=d_bf[:],
                    op0=Alu.mult, op1=Alu.mult,
                    accum_out=accz[:, b:b + 1])
                deb = step_gt5 and (b * BC < PC // 4)
                alpha = (a2 - 1.0) if deb else -SYNC
                beta = a2 if deb else (1.0 - SYNC)
                nc.vector.scalar_tensor_tensor(
                    out=q1[:], in0=zd_bf[:], scalar=beta, in1=d_bf[:],
                    op0=Alu.mult, op1=Alu.subtract)
                pa_b = ablk.tile([128, BC], bf16, tag="pab")
                nc.vector.scalar_tensor_tensor(
                    out=pa_b[:], in0=nn_bf[:], scalar=alpha, in1=q1[:],
                    op0=Alu.mult, op1=Alu.add)
                # transpose to cell-major now; v is added later in phase B
                for hh in range(2):
                    ptr = _ps3.tile([128, HB], bf16, tag="ptr")
                    for j in range(HB // 128):
                        col = hh * HB + j * 128
                        nc.tensor.transpose(
                            out=ptr[:, j * 128:(j + 1) * 128],
                            in_=pa_b[:, col:col + 128],
                            identity=ident[:])
                    dsl = slice(b * BC + hh * HB, b * BC + (hh + 1) * HB)
                    if (b + hh) % 2 == 0:
                        nc.scalar.activation(pa_cm[:, dsl], ptr[:], Act.Copy)
                    else:
                        nc.vector.tensor_copy(pa_cm[:, dsl], ptr[:])

            # ---------------- local stats (no AllReduce needed) -----------
            nc.vector.tensor_reduce(out=tmpn[:], in_=accn[:, :],
                                    axis=mybir.AxisListType.X, op=Alu.add)
            nc.vector.tensor_reduce(out=tmpv[:], in_=accz[:, :],
                                    axis=mybir.AxisListType.X, op=Alu.add)
            nc.vector.tensor_tensor(out=sums[:, 0:1], in0=tmpn[:],
                                    in1=tmpv[:], op=Alu.add)
            # v_plain = SYNC/PC * local_sum
            nc.vector.tensor_scalar(out=vvec[:, 0:1], in0=sums[:, 0:1],
                                    scalar1=SYNC / PC, scalar2=None,
                                    op0=Alu.mult)

            def emit_vcm(col, dst):
                # dst[i, j] = vvec[j, col] for all i (broadcast matrix)
                nc.vector.tensor_copy(vcol_bf[:], vvec[:, col:col + 1])
                pv1 = _ps0.tile([1, 128], f32, tag="pz", name=f"pv1_{col}")
                nc.tensor.matmul(out=pv1[:], lhsT=vcol_bf[:],
                                 rhs=ident[:], start=True, stop=True)
                nc.scalar.activation(vrow_bf[:], pv1[:], Act.Copy)
                pv2 = _ps0.tile([128, 128], f32, tag="pr", name=f"pv2_{col}")
                nc.tensor.matmul(out=pv2[:], lhsT=ones_row[0:1, 0:128],
                                 rhs=vrow_bf[:], start=True, stop=True)
                nc.scalar.activation(dst[:], pv2[:], Act.Copy)

            emit_vcm(0, v_cm_p)

            # ---------------- phase B ------------------------------------
            def emit_phase_b(b):
                deb = step_gt5 and (b * BC < PC // 4)
                vt = v_cm_d if deb else v_cm_p
                dcm = bblk.tile([128, BC], f32, tag="dcm")
                nc.vector.tensor_tensor(
                    out=dcm[:].rearrange("p (a n) -> p a n", n=H),
                    in0=pa_cm[:, b * BC:(b + 1) * BC].rearrange(
                        "p (a n) -> p a n", n=H),
                    in1=vt[:, None, :].to_broadcast([128, BC // H, H]),
                    op=Alu.add)
                if "noscatter" not in var:
                    out_ap = outsl[:, :] if b < NBLK // 2 \
                        else outsl[HALF:, :]
                    idx_ap = idxsb[:, b * (BC // 16):(b + 1) * (BC // 16)]
                    nc.gpsimd.dma_scatter_add(
                        out_ap,
                        dcm[:].rearrange("p (a n) -> p a n", n=H),
                        idx_ap,
                        BC,
                        BC,
                        H,
                    )

            ndeb = (PC // 4) // BC
            for b in range(ndeb, NBLK):
                emit_phase_b(b)

            # ---------------- global stats + AllReduce --------------------
            nc.vector.tensor_reduce(out=sums[64:65, 3:4],
                                    in_=acct[64:65, :],
                                    axis=mybir.AxisListType.X, op=Alu.add)
            # wout: bcast raw t via PE, exp on ACT (exp(bcast)=bcast(exp)),
            # multiply-accumulate on DVE at 2x (both operands SBUF bf16)
            for c in range(NCH * NBLK):
                cs = slice(c * CH, (c + 1) * CH)
                pe = _ps0.tile([64, CH], f32, tag="pz")
                nc.tensor.matmul(out=pe[:], lhsT=ones_t[64:65, 0:64],
                                 rhs=mem[64:65, cs], start=True, stop=True)
                e_sb = bblk.tile([64, CH], bf16, tag="esb")
                nc.scalar.activation(e_sb[:], pe[:], Act.Exp,
                                     accum_out=acce[0:64, c:c + 1])
                eo = bblk.tile([64, CH], bf16, tag="eo")
                nc.vector.scalar_tensor_tensor(
                    out=eo[:], in0=mem[0:64, cs], scalar=1.0, in1=e_sb[:],
                    op0=Alu.mult, op1=Alu.mult,
                    accum_out=wacc[:, c:c + 1])
            nc.vector.tensor_reduce(out=sums[0:64, 1:2], in_=wacc[:, :],
                                    axis=mybir.AxisListType.X, op=Alu.add)
            nc.vector.tensor_reduce(out=sums[0:1, 2:3], in_=acce[0:1, :],
                                    axis=mybir.AxisListType.X, op=Alu.add)

            if "nocoll" in var:
                nc.vector.tensor_scalar(out=arsb[:], in0=sums[:],
                                        scalar1=float(CORES), scalar2=None,
                                        op0=Alu.mult)
            else:
                nc.gpsimd.dma_start(out=arin[:, :], in_=sums[:])
                nc.gpsimd.collective_compute(
                    "AllReduce", Alu.add,
                    replica_groups=[list(range(CORES))],
                    ins=[arin.ap().opt()],
                    outs=[arout.ap().opt()],
                )
                nc.gpsimd.dma_start(out=arsb[:], in_=arout[:, :])
            nc.sync.dma_start(out=osmall[:, :], in_=arsb[:])

            # v_debate = SYNC*(1-DEBATE)/PC * local + DEBATE/NA * total
            nc.vector.tensor_scalar(out=tmpv[:], in0=sums[:, 0:1],
                                    scalar1=SYNC * (1.0 - DEBATE) / PC,
                                    scalar2=None, op0=Alu.mult)
            nc.vector.scalar_tensor_tensor(
                out=vvec[:, 1:2], in0=arsb[:, 0:1], scalar=DEBATE / NA,
                in1=tmpv[:], op0=Alu.mult, op1=Alu.add)
            emit_vcm(1, v_cm_d)

            for b in range(ndeb):
                emit_phase_b(b)

    nc.compile()
    return nc


def _get_graph(step_gt5: bool):
    key = (bool(step_gt5), _variant())
    if key not in _CACHE:
        _CACHE[key] = _build_graph(step_gt5)
    return _CACHE[key]


# --------------------------------------------------------------------------
# host-side sharding + launch
# --------------------------------------------------------------------------
def _wrap_idx(vals):
    """int16 index layout for dma_gather/dma_scatter_add: [128, n//16],
    idx q stored at [q % 16, q // 16], replicated to all 8 Q7 groups."""
    n = vals.shape[0]
    w = vals.reshape(n // 16, 16).T.astype(np.int16)    # [16, n//16]
    return np.tile(w, (8, 1))                            # [128, n//16]


def kernel(**inputs):
    x = np.asarray(inputs["x"], np.float32)
    hiddens = np.asarray(inputs["hiddens"], np.float32)
    Wa1 = np.asarray(inputs["Wa1"], np.float32)
    ba1 = np.asarray(inputs["ba1"], np.float32)
    Wa2 = np.asarray(inputs["Wa2"], np.float32)
    ba2 = np.asarray(inputs["ba2"], np.float32)
    Wg1 = np.asarray(inputs["Wg1"], np.float32)
    bg1 = np.asarray(inputs["bg1"], np.float32)
    Wg2 = np.asarray(inputs["Wg2"], np.float32)
    bg2 = np.asarray(inputs["bg2"], np.float32)
    W_ih = np.asarray(inputs["W_ih"], np.float32)
    W_hh = np.asarray(inputs["W_hh"], np.float32)
    b_ih = np.asarray(inputs["b_ih"], np.float32)
    b_hh = np.asarray(inputs["b_hh"], np.float32)
    ai = np.asarray(inputs["alive_idx"]).astype(np.int64)
    step = int(inputs["step"])

    ok = (
        BF16 is not None
        and hiddens.shape == (N_CELLS, H)
        and ai.shape == (NA,)
        and np.all(np.diff(ai) > 0)
        and ai[0] >= 0
        and ai[-1] < N_CELLS
        and step > 5
    )
    if ok:
        c = [0] + [int(ai[PC * k]) for k in range(1, CORES)] + [N_CELLS]
        lens = [c[k + 1] - c[k] for k in range(CORES)]
        for k in range(CORES):
            li = ai[PC * k:PC * (k + 1)] - c[k]
            if lens[k] > S or li[:HALF].max() > 32000 or \
               li[HALF:].min() < HALF or (li[HALF:] - HALF).max() > S - HALF - 1:
                ok = False
                break
    if not ok:
        return _np_reference(x, hiddens, Wa1, ba1, Wa2, ba2, Wg1, bg1, Wg2,
                             bg2, W_ih, W_hh, b_ih, b_hh, ai, step)

    # ---- replicated weight prep (x-part of layer-1 folded into biases) ----
    ba1e = ba1 + Wa1[:, :IND] @ x[0]
    bg1e = bg1 + Wg1[:, :IND] @ x[0]
    b_out = ba2 - bg2
    b_rz = (b_ih + b_hh)[:2 * H]
    b_in = b_ih[2 * H:]
    b_hn = b_hh[2 * H:]

    wbl = np.zeros((128, 1280), BF16)
    wbl[:, 0:128] = Wa1[:, IND:].T.astype(BF16)
    wbl[:, 128:256] = Wg1[:, IND:].T.astype(BF16)
    wbl[0:128, 256:320] = Wa2.T.astype(BF16)
    wbl[0:128, 320:384] = (-Wg2).T.astype(BF16)
    wbl[:, 384:512] = W_hh[0:H].T.astype(BF16)
    wbl[:, 512:640] = W_hh[H:2 * H].T.astype(BF16)
    wbl[:, 640:768] = W_hh[2 * H:].T.astype(BF16)
    wbl[0:65, 768:896] = W_ih[0:H].T.astype(BF16)
    wbl[0:65, 896:1024] = W_ih[H:2 * H].T.astype(BF16)
    wbl[0:65, 1024:1152] = W_ih[2 * H:].T.astype(BF16)
    wbl[0:1, 1152:1280] = b_hn[None, :].astype(BF16)

    bbl = np.zeros((128, 8), np.float32)
    bbl[:, 0] = ba1e
    bbl[:, 1] = bg1e
    bbl[0:64, 2] = b_out
    bbl[:, 3] = b_rz[:H]
    bbl[:, 4] = b_rz[H:]
    bbl[:, 5] = b_in

    in_maps = []
    for k in range(CORES):
        lo = c[k]
        sl = hiddens[lo:lo + S]
        if sl.shape[0] < S:
            sl = np.concatenate(
                [sl, np.zeros((S - sl.shape[0], H), np.float32)], axis=0)
        li = (ai[PC * k:PC * (k + 1)] - lo).astype(np.int64)
        idxw = np.zeros((128, NBLK * (BC // 16)), np.int16)
        for b in range(NBLK):
            vals = li[b * BC:(b + 1) * BC].copy()
            if b >= NBLK // 2:
                vals -= HALF
            idxw[:, b * (BC // 16):(b + 1) * (BC // 16)] = _wrap_idx(vals)
        in_maps.append({
            "hslf": np.ascontiguousarray(sl),
            "hslb": np.ascontiguousarray(sl.astype(BF16)),
            "idxw": idxw,
            "wbl": wbl,
            "bbl": bbl,
        })

    nc = _get_graph(step > 5)
    from concourse.bass_utils import run_bass_kernel_spmd
    res = run_bass_kernel_spmd(nc, in_maps, core_ids=list(range(CORES)))
    kernel._last_result = res
    kernel._last_in_maps = in_maps

    new_hiddens = np.empty((N_CELLS, H), np.float32)
    for k in range(CORES):
        new_hiddens[c[k]:c[k + 1]] = res.results[k]["outsl"][:c[k + 1] - c[k]]
    stats = res.results[0]["osmall"]
    wout = stats[0:64, 1]
    sum_e = stats[0, 2]
    sum_t = stats[64, 3]
    combined = (wout / sum_e).astype(np.float32)[None, :]
    mean_tension = np.float32(sum_t / NA)
    return combined, mean_tension, new_hiddens


kernel._last_result = None
kernel._last_in_maps = None


# revision 40
# speedup vs baseline: 2.6608x; 2.0887x over previous
"""Trainium2 Bass kernel for nn_AutopoieticEngine (scatter_memory).

Self-contained: takes FULL inputs (as produced by the problem's
setup_inputs), shards the cell dimension across 8 NeuronCores, runs a
Bass/Tile kernel per core (gather -> MLP+GRU -> faction sync/debate ->
scatter), all-reduces the tiny faction/softmax statistics on-device, and
reassembles the full outputs on the host.

Sharding: alive positions split into 8 chunks of 16384 = exactly one
faction per core (Na=131072, n_f=8, fs=16384).  Core k owns hiddens rows
[c_k, c_{k+1}) where c_k = alive_idx[16384*k], so its scatter targets are
entirely inside its own (padded) row slice.
"""

import numpy as np

try:
    import ml_dtypes

    BF16 = ml_dtypes.bfloat16
except Exception:  # pragma: no cover
    BF16 = None

CORES = 8
H = 128          # hidden dim
IND = 64         # input dim
D = 64           # out dim
N_CELLS = 262144
NA = 131072
PC = NA // CORES      # alive positions per core == faction size
S = 33792             # padded hiddens-slice rows per core (264*128)
HALF = 8192           # int16-index base split point (positions per core)
NBLK = 8              # cell blocks per core
BC = PC // NBLK       # 2048 cells per block
NCH = BC // 512       # 512-wide matmul chunks per block
CH = 512
SYNC = 0.15
DEBATE = 0.15
CPR = 4224            # copy-pass rows per tile (S/8, multiple of 128)

_CACHE = {}


def _variant():
    import os
    return frozenset(
        v for v in os.environ.get("KVAR", "").split(",") if v)


# --------------------------------------------------------------------------
# numpy fallback (exact reference semantics) for inputs that violate the
# layout assumptions this kernel hardcodes.
# --------------------------------------------------------------------------
def _np_reference(x, hiddens, Wa1, ba1, Wa2, ba2, Wg1, bg1, Wg2, bg2,
                  W_ih, W_hh, b_ih, b_hh, alive_idx, step):
    idx = np.asarray(alive_idx).astype(np.int64)
    h = hiddens[idx]
    xb = np.broadcast_to(x, (h.shape[0], x.shape[-1]))
    c = np.concatenate([xb, h], axis=-1)
    a = np.maximum(c @ Wa1.T + ba1, 0) @ Wa2.T + ba2
    g = np.maximum(c @ Wg1.T + bg1, 0) @ Wg2.T + bg2
    out = a - g
    tension = np.mean(out * out, axis=-1, keepdims=True)
    mem_in = np.concatenate([out, tension], axis=-1)
    gi = mem_in @ W_ih.T + b_ih
    gh = h @ W_hh.T + b_hh
    Hd = h.shape[-1]
    ir, iz, inn = gi[:, :Hd], gi[:, Hd:2 * Hd], gi[:, 2 * Hd:]
    hr, hz, hn = gh[:, :Hd], gh[:, Hd:2 * Hd], gh[:, 2 * Hd:]
    r = 1.0 / (1.0 + np.exp(-(ir + hr)))
    z = 1.0 / (1.0 + np.exp(-(iz + hz)))
    nn_ = np.tanh(inn + r * hn)
    new_h = (1.0 - z) * nn_ + z * h
    n, Hh = new_h.shape
    n_f = min(8, n // 2)
    if n_f >= 2:
        fs = n // n_f
        hb = new_h[: n_f * fs].reshape(n_f, fs, Hh)
        fm = hb.mean(axis=1, keepdims=True)
        hb = (1.0 - SYNC) * hb + SYNC * fm
        if step > 5:
            go = hb.mean(axis=1).mean(axis=0)
            dc = max(1, fs // 4)
            hb[:, :dc] = (1.0 - DEBATE) * hb[:, :dc] + DEBATE * go
        new_h = np.concatenate([hb.reshape(n_f * fs, Hh), new_h[n_f * fs:]], 0)
    new_hiddens = np.asarray(hiddens).copy()
    new_hiddens[idx] = new_h
    t = tension[:, 0]
    tm = t.max()
    w = np.exp(t - tm)
    w = w / w.sum()
    combined = (w[:, None] * out).sum(axis=0, keepdims=True)
    mean_tension = np.float32(t.mean())
    return (combined.astype(np.float32), mean_tension,
            new_hiddens.astype(np.float32))


# --------------------------------------------------------------------------
# graph builder
# --------------------------------------------------------------------------
def _build_graph(step_gt5: bool):
    import concourse.bass as bass
    import concourse.mybir as mybir
    import concourse.tile as tile
    from concourse import bacc
    from concourse.masks import make_identity

    var = _variant()

    f32 = mybir.dt.float32
    bf16 = mybir.dt.bfloat16
    i16 = mybir.dt.int16
    Alu = mybir.AluOpType
    Act = mybir.ActivationFunctionType

    nc = bacc.Bacc("TRN2", target_bir_lowering=False, debug=False,
                   num_devices=CORES)

    hslf = nc.declare_dram_parameter("hslf", [S, H], f32, isOutput=False)
    hslb = nc.declare_dram_parameter("hslb", [S, H], bf16, isOutput=False)
    idxw = nc.declare_dram_parameter("idxw", [128, NBLK * (BC // 16)], i16,
                                     isOutput=False)
    wbl = nc.declare_dram_parameter("wbl", [128, 1280], bf16, isOutput=False)
    bbl = nc.declare_dram_parameter("bbl", [128, 8], f32, isOutput=False)
    outsl = nc.declare_dram_parameter("outsl", [S, H], f32, isOutput=True)
    osmall = nc.declare_dram_parameter("osmall", [128, 4], f32, isOutput=True)

    arin = nc.dram_tensor("arin", [128, 4], f32)
    arout = nc.dram_tensor("arout", [128, 4], f32, addr_space="Shared")

    # constants for the fused sync/debate delta:
    #   delta = alpha*nn + beta*zd - d + v
    # non-debate: alpha=-SYNC, beta=1-SYNC, v=SYNC*fm
    # debate:     a2=(1-SYNC)(1-DEBATE); alpha=a2-1, beta=a2,
    #             v=SYNC*(1-DEBATE)*fm + DEBATE*go
    a2 = (1.0 - SYNC) * (1.0 - DEBATE)

    with tile.TileContext(nc) as tc:
        with (
            tc.tile_pool(name="const", bufs=1) as cpool,
            tc.tile_pool(name="per", bufs=1) as per,
            tc.tile_pool(name="ablk", bufs=1) as ablk,
            tc.tile_pool(name="h2", bufs=3) as h2,
            tc.tile_pool(name="bblk", bufs=2) as bblk,
            tc.tile_pool(name="cp", bufs=2) as cp,
            tc.tile_pool(name="ps", bufs=1, space="PSUM") as _ps0,
            tc.tile_pool(name="ps2", bufs=2, space="PSUM") as _ps2,
            tc.tile_pool(name="ps3", bufs=2, space="PSUM") as _ps3,
        ):
            HB = BC // 2  # 1024: half-block, one wide-psum tile
            # ---------------- constant / persistent tiles ----------------
            wsb = cpool.tile([128, 1280], bf16)
            bsb = cpool.tile([128, 8], f32)
            idxsb = cpool.tile([128, NBLK * (BC // 16)], i16)
            ident = cpool.tile([128, 128], bf16)
            ones_row = cpool.tile([1, CH], bf16)      # rhs for b_hn outer
            ones64 = cpool.tile([64, 65], bf16)       # tension lhsT (1/64)
            ones_t = cpool.tile([65, 64], bf16)       # e-bcast lhsT @ base 64

            nc.sync.dma_start(out=wsb[:], in_=wbl[:, :])
            nc.sync.dma_start(out=bsb[:], in_=bbl[:, :])
            nc.sync.dma_start(out=idxsb[:], in_=idxw[:, :])
            make_identity(nc, ident[:])
            nc.gpsimd.memset(ones_row[:], 1.0)
            nc.gpsimd.memset(ones64[:], 1.0 / 64.0)
            nc.gpsimd.memset(ones_t[:], 1.0)

            pa_cm = per.tile([128, PC], bf16)  # cell-major pa (transposed)
            mem = per.tile([65, PC], bf16)     # rows 0:64 out, row 64 t
            accn = per.tile([128, NBLK], f32)  # sum(nn) per block
            accz = per.tile([128, NBLK], f32)  # sum(z*d) per block
            acct = per.tile([65, NCH * NBLK], f32)  # row 64: sum(t) slots
            acce = per.tile([64, NCH * NBLK], f32)  # row0: sum(e) slots
            wacc = per.tile([64, NCH * NBLK], f32)  # wout partials
            sums = per.tile([128, 4], f32)     # AllReduce payload
            arsb = per.tile([128, 4], f32)     # AllReduce result
            vvec = per.tile([128, 2], f32)     # v (plain, debate)
            tmpv = per.tile([128, 1], f32)
            tmpn = per.tile([128, 1], f32)
            v_cm_p = per.tile([128, 128], bf16)   # v broadcast, plain
            v_cm_d = per.tile([128, 128], bf16)   # v broadcast, debate
            vcol_bf = per.tile([128, 1], bf16)
            vrow_bf = per.tile([1, 128], bf16)

            nc.gpsimd.memset(sums[:], 0.0)

            # ---------------- phase A ------------------------------------
            def emit_gather(b, h_tile):
                if "nogather" in var:
                    nc.gpsimd.memset(h_tile[:], 0.02)
                    return
                in_ap = hslb[:, :] if b < NBLK // 2 else hslb[HALF:, :]
                # transpose-mode dma_gather is limited to 512 idxs/call
                for c in range(NCH):
                    nc.gpsimd.dma_gather(
                        h_tile[:, c * CH:(c + 1) * CH].rearrange(
                            "p (a n) -> p a n", a=1),
                        in_ap,
                        idxsb[:, (b * NCH + c) * (CH // 16):
                              (b * NCH + c + 1) * (CH // 16)],
                        CH,
                        CH,
                        H,
                        transpose=True,
                    )

            h_tiles = {}
            for pb in (0, 1):
                h_tiles[pb] = h2.tile([128, BC], bf16, tag="h",
                                      name=f"hbf{pb}")
                emit_gather(pb, h_tiles[pb])

            for b in range(NBLK):
                blk = slice(b * BC, (b + 1) * BC)
                h_bf = h_tiles.pop(b)
                if b + 2 < NBLK:
                    h_tiles[b + 2] = h2.tile([128, BC], bf16, tag="h",
                                             name=f"hbf{b+2}")
                    emit_gather(b + 2, h_tiles[b + 2])

                a1 = ablk.tile([128, BC], bf16, tag="a1")
                g1 = ablk.tile([128, BC], bf16, tag="g1")
                sq = ablk.tile([64, BC], bf16, tag="sq")
                r_bf = ablk.tile([128, BC], bf16, tag="r")
                z_bf = ablk.tile([128, BC], bf16, tag="z")
                rhn = ablk.tile([128, BC], bf16, tag="rhn")
                s_bf = ablk.tile([128, BC], bf16, tag="s")
                nn_bf = ablk.tile([128, BC], bf16, tag="nn")
                d_bf = ablk.tile([128, BC], bf16, tag="d")
                zd_bf = ablk.tile([128, BC], bf16, tag="zd")
                q1 = ablk.tile([128, BC], bf16, tag="q1")

                # stage 1/2: first-layer MLPs (x-part folded into bias)
                for c in range(NCH):
                    cs = slice(c * CH, (c + 1) * CH)
                    p1 = _ps2.tile([128, CH], f32, tag="p1")
                    nc.tensor.matmul(out=p1[:], lhsT=wsb[:, 0:128],
                                     rhs=h_bf[:, cs], start=True, stop=True)
                    nc.scalar.activation(a1[:, cs], p1[:], Act.Relu,
                                         bias=bsb[:, 0:1])
                for c in range(NCH):
                    cs = slice(c * CH, (c + 1) * CH)
                    p1 = _ps2.tile([128, CH], f32, tag="p1")
                    nc.tensor.matmul(out=p1[:], lhsT=wsb[:, 128:256],
                                     rhs=h_bf[:, cs], start=True, stop=True)
                    nc.scalar.activation(g1[:, cs], p1[:], Act.Relu,
                                         bias=bsb[:, 1:2])

                # stage 3: out = Wa2@a1 - Wg2@g1 + b_out  -> mem rows 0:64
                for c in range(NCH):
                    cs = slice(c * CH, (c + 1) * CH)
                    mcs = slice(b * BC + c * CH, b * BC + (c + 1) * CH)
                    po = _ps2.tile([64, CH], f32, tag="p1")
                    nc.tensor.matmul(out=po[:], lhsT=wsb[0:128, 256:320],
                                     rhs=a1[:, cs], start=True, stop=False)
                    nc.tensor.matmul(out=po[:], lhsT=wsb[0:128, 320:384],
                                     rhs=g1[:, cs], start=False, stop=True)
                    nc.scalar.activation(mem[0:64, mcs], po[:], Act.Identity,
                                         bias=bsb[0:64, 2:3])

                # tension: t = mean(out^2) over features -> mem row 64
                nc.vector.tensor_tensor(out=sq[:], in0=mem[0:64, blk],
                                        in1=mem[0:64, blk], op=Alu.mult)
                for c in range(NCH):
                    cs = slice(c * CH, (c + 1) * CH)
                    mcs = slice(b * BC + c * CH, b * BC + (c + 1) * CH)
                    pt = _ps2.tile([65, CH], f32, tag="p1")
                    nc.tensor.matmul(out=pt[:, :], lhsT=ones64[:, :],
                                     rhs=sq[:, cs], start=True, stop=True)
                    nc.scalar.activation(
                        mem[64:65, mcs], pt[64:65, :], Act.Copy,
                        accum_out=acct[64:65, b * NCH + c:b * NCH + c + 1])

                # GRU gates
                for c in range(NCH):
                    cs = slice(c * CH, (c + 1) * CH)
                    mcs = slice(b * BC + c * CH, b * BC + (c + 1) * CH)
                    pr = _ps0.tile([128, CH], f32, tag="pr")
                    pz = _ps0.tile([128, CH], f32, tag="pz")
                    pi = _ps0.tile([128, CH], f32, tag="pi")
                    ph = _ps0.tile([128, CH], f32, tag="ph")
                    nc.tensor.matmul(out=pr[:], lhsT=wsb[0:65, 768:896],
                                     rhs=mem[0:65, mcs], start=True,
                                     stop=False)
                    nc.tensor.matmul(out=pr[:], lhsT=wsb[:, 384:512],
                                     rhs=h_bf[:, cs], start=False, stop=True)
                    nc.tensor.matmul(out=pz[:], lhsT=wsb[0:65, 896:1024],
                                     rhs=mem[0:65, mcs], start=True,
                                     stop=False)
                    nc.tensor.matmul(out=pz[:], lhsT=wsb[:, 512:640],
                                     rhs=h_bf[:, cs], start=False, stop=True)
                    nc.tensor.matmul(out=pi[:], lhsT=wsb[0:65, 1024:1152],
                                     rhs=mem[0:65, mcs], start=True,
                                     stop=True)
                    nc.tensor.matmul(out=ph[:], lhsT=wsb[0:1, 1152:1280],
                                     rhs=ones_row[:, :], start=True,
                                     stop=False)
                    nc.tensor.matmul(out=ph[:], lhsT=wsb[:, 640:768],
                                     rhs=h_bf[:, cs], start=False, stop=True)
                    nc.scalar.activation(r_bf[:, cs], pr[:], Act.Sigmoid,
                                         bias=bsb[:, 3:4])
                    nc.scalar.activation(z_bf[:, cs], pz[:], Act.Sigmoid,
                                         bias=bsb[:, 4:5])
                    nc.vector.tensor_tensor(out=rhn[:, cs], in0=r_bf[:, cs],
                                            in1=ph[:], op=Alu.mult)
                    nc.vector.tensor_tensor(out=s_bf[:, cs], in0=rhn[:, cs],
                                            in1=pi[:], op=Alu.add)

                nc.scalar.activation(nn_bf[:], s_bf[:], Act.Tanh,
                                     bias=bsb[:, 5:6],
                                     accum_out=accn[:, b:b + 1])
                nc.vector.tensor_tensor(out=d_bf[:], in0=h_bf[:],
                                        in1=nn_bf[:], op=Alu.subtract)
                nc.vector.scalar_tensor_tensor(
                    out=zd_bf[:], in0=z_bf[:], scalar=1.0, in1=d_bf[:],
                    op0=Alu.mult, op1=Alu.mult,
                    accum_out=accz[:, b:b + 1])
                deb = step_gt5 and (b * BC < PC // 4)
                alpha = (a2 - 1.0) if deb else -SYNC
                beta = a2 if deb else (1.0 - SYNC)
                nc.vector.scalar_tensor_tensor(
                    out=q1[:], in0=zd_bf[:], scalar=beta, in1=d_bf[:],
                    op0=Alu.mult, op1=Alu.subtract)
                pa_b = ablk.tile([128, BC], bf16, tag="pab")
                nc.vector.scalar_tensor_tensor(
                    out=pa_b[:], in0=nn_bf[:], scalar=alpha, in1=q1[:],
                    op0=Alu.mult, op1=Alu.add)
                # transpose to cell-major now; v is added later in phase B
                for hh in range(2):
                    ptr = _ps3.tile([128, HB], bf16, tag="ptr")
                    for j in range(HB // 128):
                        col = hh * HB + j * 128
                        nc.tensor.transpose(
                            out=ptr[:, j * 128:(j + 1) * 128],
                            in_=pa_b[:, col:col + 128],
                            identity=ident[:])
                    dsl = slice(b * BC + hh * HB, b * BC + (hh + 1) * HB)
                    if (b + hh) % 2 == 0:
                        nc.scalar.activation(pa_cm[:, dsl], ptr[:], Act.Copy)
                    else:
                        nc.vector.tensor_copy(pa_cm[:, dsl], ptr[:])

            # ---------------- pass-through copy of the slice --------------
            for i in range(S // CPR):
                ct = cp.tile([128, CPR], f32, tag="cp")
                src = hslf[i * CPR:(i + 1) * CPR, :].rearrange(
                    "(p a) d -> p (a d)", p=128)
                dst = outsl[i * CPR:(i + 1) * CPR, :].rearrange(
                    "(p a) d -> p (a d)", p=128)
                nc.sync.dma_start(out=ct[:], in_=src)
                nc.sync.dma_start(out=dst, in_=ct[:])

            # ---------------- local stats (no AllReduce needed) -----------
            nc.vector.tensor_reduce(out=tmpn[:], in_=accn[:, :],
                                    axis=mybir.AxisListType.X, op=Alu.add)
            nc.vector.tensor_reduce(out=tmpv[:], in_=accz[:, :],
                                    axis=mybir.AxisListType.X, op=Alu.add)
            nc.vector.tensor_tensor(out=sums[:, 0:1], in0=tmpn[:],
                                    in1=tmpv[:], op=Alu.add)
            # v_plain = SYNC/PC * local_sum
            nc.vector.tensor_scalar(out=vvec[:, 0:1], in0=sums[:, 0:1],
                                    scalar1=SYNC / PC, scalar2=None,
                                    op0=Alu.mult)

            def emit_vcm(col, dst):
                # dst[i, j] = vvec[j, col] for all i (broadcast matrix)
                nc.vector.tensor_copy(vcol_bf[:], vvec[:, col:col + 1])
                pv1 = _ps0.tile([1, 128], f32, tag="pz", name=f"pv1_{col}")
                nc.tensor.matmul(out=pv1[:], lhsT=vcol_bf[:],
                                 rhs=ident[:], start=True, stop=True)
                nc.scalar.activation(vrow_bf[:], pv1[:], Act.Copy)
                pv2 = _ps0.tile([128, 128], f32, tag="pr", name=f"pv2_{col}")
                nc.tensor.matmul(out=pv2[:], lhsT=ones_row[0:1, 0:128],
                                 rhs=vrow_bf[:], start=True, stop=True)
                nc.scalar.activation(dst[:], pv2[:], Act.Copy)

            emit_vcm(0, v_cm_p)

            # ---------------- phase B ------------------------------------
            def emit_phase_b(b):
                deb = step_gt5 and (b * BC < PC // 4)
                vt = v_cm_d if deb else v_cm_p
                dcm = bblk.tile([128, BC], f32, tag="dcm")
                nc.vector.tensor_tensor(
                    out=dcm[:].rearrange("p (a n) -> p a n", n=H),
                    in0=pa_cm[:, b * BC:(b + 1) * BC].rearrange(
                        "p (a n) -> p a n", n=H),
                    in1=vt[:, None, :].to_broadcast([128, BC // H, H]),
                    op=Alu.add)
                if "noscatter" not in var:
                    out_ap = outsl[:, :] if b < NBLK // 2 \
                        else outsl[HALF:, :]
                    idx_ap = idxsb[:, b * (BC // 16):(b + 1) * (BC // 16)]
                    nc.gpsimd.dma_scatter_add(
                        out_ap,
                        dcm[:].rearrange("p (a n) -> p a n", n=H),
                        idx_ap,
                        BC,
                        BC,
                        H,
                    )

            ndeb = (PC // 4) // BC
            for b in range(ndeb, NBLK):
                emit_phase_b(b)

            # ---------------- global stats + AllReduce --------------------
            nc.vector.tensor_reduce(out=sums[64:65, 3:4],
                                    in_=acct[64:65, :],
                                    axis=mybir.AxisListType.X, op=Alu.add)
            # wout: bcast raw t via PE, exp on ACT (exp(bcast)=bcast(exp)),
            # multiply-accumulate on DVE at 2x (both operands SBUF bf16)
            for c in range(NCH * NBLK):
                cs = slice(c * CH, (c + 1) * CH)
                pe = _ps0.tile([64, CH], f32, tag="pz" if c % 2 else "pi")
                nc.tensor.matmul(out=pe[:], lhsT=ones_t[64:65, 0:64],
                                 rhs=mem[64:65, cs], start=True, stop=True)
                e_sb = bblk.tile([64, CH], bf16, tag="esb")
                nc.scalar.activation(e_sb[:], pe[:], Act.Exp,
                                     accum_out=acce[0:64, c:c + 1])
                eo = bblk.tile([64, CH], bf16, tag="eo")
                nc.vector.scalar_tensor_tensor(
                    out=eo[:], in0=mem[0:64, cs], scalar=1.0, in1=e_sb[:],
                    op0=Alu.mult, op1=Alu.mult,
                    accum_out=wacc[:, c:c + 1])
            nc.vector.tensor_reduce(out=sums[0:64, 1:2], in_=wacc[:, :],
                                    axis=mybir.AxisListType.X, op=Alu.add)
            nc.vector.tensor_reduce(out=sums[0:1, 2:3], in_=acce[0:1, :],
                                    axis=mybir.AxisListType.X, op=Alu.add)

            if "nocoll" in var:
                nc.vector.tensor_scalar(out=arsb[:], in0=sums[:],
                                        scalar1=float(CORES), scalar2=None,
                                        op0=Alu.mult)
            else:
                nc.gpsimd.dma_start(out=arin[:, :], in_=sums[:])
                nc.gpsimd.collective_compute(
                    "AllReduce", Alu.add,
                    replica_groups=[list(range(CORES))],
                    ins=[arin.ap().opt()],
                    outs=[arout.ap().opt()],
                )
                nc.gpsimd.dma_start(out=arsb[:], in_=arout[:, :])
            nc.sync.dma_start(out=osmall[:, :], in_=arsb[:])

            # v_debate = SYNC*(1-DEBATE)/PC * local + DEBATE/NA * total
            nc.vector.tensor_scalar(out=tmpv[:], in0=sums[:, 0:1],
                                    scalar1=SYNC * (1.0 - DEBATE) / PC,
                                    scalar2=None, op0=Alu.mult)
            nc.vector.scalar_tensor_tensor(
                out=vvec[:, 1:2], in0=arsb[:, 0:1], scalar=DEBATE / NA,
                in1=tmpv[:], op0=Alu.mult, op1=Alu.add)
            emit_vcm(1, v_cm_d)

            for b in range(ndeb):
                emit_phase_b(b)

    nc.compile()
    return nc


def _get_graph(step_gt5: bool):
    key = (bool(step_gt5), _variant())
    if key not in _CACHE:
        _CACHE[key] = _build_graph(step_gt5)
    return _CACHE[key]


# --------------------------------------------------------------------------
# host-side sharding + launch
# --------------------------------------------------------------------------
def _wrap_idx(vals):
    """int16 index layout for dma_gather/dma_scatter_add: [128, n//16],
    idx q stored at [q % 16, q // 16], replicated to all 8 Q7 groups."""
    n = vals.shape[0]
    w = vals.reshape(n // 16, 16).T.astype(np.int16)    # [16, n//16]
    return np.tile(w, (8, 1))                            # [128, n//16]


def kernel(**inputs):
    x = np.asarray(inputs["x"], np.float32)
    hiddens = np.asarray(inputs["hiddens"], np.float32)
    Wa1 = np.asarray(inputs["Wa1"], np.float32)
    ba1 = np.asarray(inputs["ba1"], np.float32)
    Wa2 = np.asarray(inputs["Wa2"], np.float32)
    ba2 = np.asarray(inputs["ba2"], np.float32)
    Wg1 = np.asarray(inputs["Wg1"], np.float32)
    bg1 = np.asarray(inputs["bg1"], np.float32)
    Wg2 = np.asarray(inputs["Wg2"], np.float32)
    bg2 = np.asarray(inputs["bg2"], np.float32)
    W_ih = np.asarray(inputs["W_ih"], np.float32)
    W_hh = np.asarray(inputs["W_hh"], np.float32)
    b_ih = np.asarray(inputs["b_ih"], np.float32)
    b_hh = np.asarray(inputs["b_hh"], np.float32)
    ai = np.asarray(inputs["alive_idx"]).astype(np.int64)
    step = int(inputs["step"])

    ok = (
        BF16 is not None
        and hiddens.shape == (N_CELLS, H)
        and ai.shape == (NA,)
        and np.all(np.diff(ai) > 0)
        and ai[0] >= 0
        and ai[-1] < N_CELLS
        and step > 5
    )
    if ok:
        c = [0] + [int(ai[PC * k]) for k in range(1, CORES)] + [N_CELLS]
        lens = [c[k + 1] - c[k] for k in range(CORES)]
        for k in range(CORES):
            li = ai[PC * k:PC * (k + 1)] - c[k]
            if lens[k] > S or li[:HALF].max() > 32000 or \
               li[HALF:].min() < HALF or (li[HALF:] - HALF).max() > S - HALF - 1:
                ok = False
                break
    if not ok:
        return _np_reference(x, hiddens, Wa1, ba1, Wa2, ba2, Wg1, bg1, Wg2,
                             bg2, W_ih, W_hh, b_ih, b_hh, ai, step)

    # ---- replicated weight prep (x-part of layer-1 folded into biases) ----
    ba1e = ba1 + Wa1[:, :IND] @ x[0]
    bg1e = bg1 + Wg1[:, :IND] @ x[0]
    b_out = ba2 - bg2
    b_rz = (b_ih + b_hh)[:2 * H]
    b_in = b_ih[2 * H:]
    b_hn = b_hh[2 * H:]

    wbl = np.zeros((128, 1280), BF16)
    wbl[:, 0:128] = Wa1[:, IND:].T.astype(BF16)
    wbl[:, 128:256] = Wg1[:, IND:].T.astype(BF16)
    wbl[0:128, 256:320] = Wa2.T.astype(BF16)
    wbl[0:128, 320:384] = (-Wg2).T.astype(BF16)
    wbl[:, 384:512] = W_hh[0:H].T.astype(BF16)
    wbl[:, 512:640] = W_hh[H:2 * H].T.astype(BF16)
    wbl[:, 640:768] = W_hh[2 * H:].T.astype(BF16)
    wbl[0:65, 768:896] = W_ih[0:H].T.astype(BF16)
    wbl[0:65, 896:1024] = W_ih[H:2 * H].T.astype(BF16)
    wbl[0:65, 1024:1152] = W_ih[2 * H:].T.astype(BF16)
    wbl[0:1, 1152:1280] = b_hn[None, :].astype(BF16)

    bbl = np.zeros((128, 8), np.float32)
    bbl[:, 0] = ba1e
    bbl[:, 1] = bg1e
    bbl[0:64, 2] = b_out
    bbl[:, 3] = b_rz[:H]
    bbl[:, 4] = b_rz[H:]
    bbl[:, 5] = b_in

    in_maps = []
    for k in range(CORES):
        lo = c[k]
        sl = hiddens[lo:lo + S]
        if sl.shape[0] < S:
            sl = np.concatenate(
                [sl, np.zeros((S - sl.shape[0], H), np.float32)], axis=0)
        li = (ai[PC * k:PC * (k + 1)] - lo).astype(np.int64)
        idxw = np.zeros((128, NBLK * (BC // 16)), np.int16)
        for b in range(NBLK):
            vals = li[b * BC:(b + 1) * BC].copy()
            if b >= NBLK // 2:
                vals -= HALF
            idxw[:, b * (BC // 16):(b + 1) * (BC // 16)] = _wrap_idx(vals)
        in_maps.append({
            "hslf": np.ascontiguousarray(sl),
            "hslb": np.ascontiguousarray(sl.astype(BF16)),
            "idxw": idxw,
            "wbl": wbl,
            "bbl": bbl,
        })

    nc = _get_graph(step > 5)
    from concourse.bass_utils import run_bass_kernel_spmd
    res = run_bass_kernel_spmd(nc, in_maps, core_ids=list(range(CORES)))
    kernel._last_result = res
    kernel._last_in_maps = in_maps

    new_hiddens = np.empty((N_CELLS, H), np.float32)
    for k in range(CORES):
        new_hiddens[c[k]:c[k + 1]] = res.results[k]["outsl"][:c[k + 1] - c[k]]
    stats = res.results[0]["osmall"]
    wout = stats[0:64, 1]
    sum_e = stats[0, 2]
    sum_t = stats[64, 3]
    combined = (wout / sum_e).astype(np.float32)[None, :]
    mean_tension = np.float32(sum_t / NA)
    return combined, mean_tension, new_hiddens


kernel._last_result = None
kernel._last_in_maps = None
